# revision 6
# baseline (speedup 1.0000x reference)
"""Trainium2 Bass kernel for nn_CartographerPoseCorrector (v2: fp8 DoubleRow).

Same algorithm as v1 (moment-correlation surfaces on TensorE + exact host
rescore of a margin set), restructured for speed:

- fp8e4m3 inputs + DoubleRow matmuls: contraction pairs the two halves of
  the 176-column canvas window, so one matmul contracts all columns.
- Lag windows trimmed to the candidate range actually reachable
  (48 x 48 lags, J-window 49).
- The (theta', m) axes are interleaved *inside* the canvas-row axis of the
  splat tensor, so each accumulation step's J-window is one contiguous
  392-element slice -> a legal 3-dim DoubleRow rhs AP.
- The nbr window tensor arrives from the host already repacked
  (no on-chip DVE repack).
- Dummy warm-up matmuls run during the input DMA to lift the PE HAM
  clock-gate before the real accumulation starts.

Geometry (device):
  W[ki, half, i, 2t+slot] = nbr[2i+slot, c + t - 68],  c = 21 + ki + 88*half
  X[ki, half, g, r', 4*tp+m] = S_{2g+tp}[m, 195 - r', c]
  psum[g][2t+slot, j*8 + tp*4 + m]
      += sum_{ki,half,i} W[ki,half,i,2t+slot] * X[ki,half,g,126-2i+j, ...]
  => T_m,theta[K=t-24, J=j+slot-25] after the host fold.
"""

import math
import sys

import numpy as np

H = W = 128
THRESH = 0.3
TRANS_RANGE = 20.0
ROT_RANGE = 15.0
COARSE_STEP = 2.0
FINE_STEP = 0.5

# Device-kernel geometry (must match the Bass program)
CANVAS = 224     # splat canvas extent (host-side bounds check)
OFF = 44         # image coord -> canvas coord offset
NL = 48          # lags per axis
LMIN = -24       # lag range [LMIN, LMIN + NL)
NJ = NL + 1      # J-window width per slot
MMQ = 2 * NL     # weight columns (= psum partitions)
KI = 88          # contraction partitions (x2 halves = 176 canvas cols)
CL0 = 21         # first canvas column on device
XOFF2 = 68       # nbr col = c + t - XOFF2
R0 = 126         # window offset: roff = R0 - 2*i
ROWBASE = 195    # S row = ROWBASE - r'
RP = 176         # r' extent
G = 2            # theta groups per core (2 thetas each)
PF = 8           # (theta', m) interleave factor inside r'
NI = H // 2      # accumulation steps
M = 4            # moments
U = 4            # thetas per core
N_CORES = 8
NFREE = NJ * PF  # 392: psum free size

DELTA_COARSE = 320.0   # exact-rescore safety margin
RESCORE_CAP = 2800     # hard cap on rescored coarse candidates per pair

_NC = None
LAST_MAPS = None  # debug/timing aid: last device input maps (fp8-converted)
LAST_TOUTS = None  # debug aid: last raw device outputs


# ----------------------------------------------------------------------------
# host math (mirrors reference numerics in fp32 where it matters)
# ----------------------------------------------------------------------------

def _grid_1d(align_corners):
    if align_corners:
        xs = np.linspace(-1.0, 1.0, W, dtype=np.float32)
        ys = np.linspace(-1.0, 1.0, H, dtype=np.float32)
    else:
        xs = ((2.0 * np.arange(W, dtype=np.float32) + 1.0) / W - 1.0)
        ys = ((2.0 * np.arange(H, dtype=np.float32) + 1.0) / H - 1.0)
    return xs, ys


def _coarse_cands():
    dxs = np.arange(-TRANS_RANGE, TRANS_RANGE + 1e-3, COARSE_STEP, dtype=np.float32)
    drs = np.arange(-ROT_RANGE, ROT_RANGE + 1e-3, COARSE_STEP, dtype=np.float32)
    gdx, gdy, gdr = np.meshgrid(dxs, dxs, drs, indexing="ij")
    return np.stack([gdx.ravel(), gdy.ravel(), gdr.ravel()], axis=1)


def _fine_cands(cp):
    off = np.arange(-COARSE_STEP, COARSE_STEP + 1e-3, FINE_STEP, dtype=np.float32)
    gdx, gdy, gdr = np.meshgrid(cp[0] + off, cp[1] + off, cp[2] + off, indexing="ij")
    return np.stack([gdx.ravel(), gdy.ravel(), gdr.ravel()], axis=1)


def _cand_affines(cands, base_2x3):
    dx, dy, dr = cands[:, 0], cands[:, 1], cands[:, 2]
    tx = (2.0 * dx / max(W - 1, 1)).astype(np.float32)
    ty = (2.0 * dy / max(H - 1, 1)).astype(np.float32)
    th = (dr * np.float32(math.pi / 180.0)).astype(np.float32)
    c, s = np.cos(th), np.sin(th)
    z, o = np.zeros_like(c), np.ones_like(c)
    delta = np.stack([c, -s, tx, s, c, ty, z, z, o], axis=-1).reshape(-1, 3, 3)
    base3 = np.concatenate([base_2x3, np.array([[0, 0, 1]], np.float32)], axis=0)
    return np.einsum("ij,njk->nik", base3.astype(np.float32), delta.astype(np.float32))[
        :, :2, :
    ].astype(np.float32)


def _pad_nbr(nbr_c, padb=8):
    out = np.zeros((H + 2 * padb, W + 2 * padb), np.float32)
    out[padb : padb + H, padb : padb + W] = nbr_c
    return out


def _exact_scores(ego_c, nbrP, affs, align_corners, padb=8, chunk=16):
    """Exact fp32 bilinear grid-sample scores for candidate affines [n,2,3]."""
    xs, ys = _grid_1d(align_corners)
    gx = np.broadcast_to(xs[None, :], (H, W)).ravel().astype(np.float32)
    gy = np.broadcast_to(ys[:, None], (H, W)).ravel().astype(np.float32)
    flat = nbrP.ravel()
    Wp = nbrP.shape[1]
    if align_corners:
        scx, ox = np.float32(0.5 * (W - 1)), np.float32(0.5 * (W - 1))
        scy, oy = np.float32(0.5 * (H - 1)), np.float32(0.5 * (H - 1))
    else:
        scx, ox = np.float32(0.5 * W), np.float32(0.5 * W - 0.5)
        scy, oy = np.float32(0.5 * H), np.float32(0.5 * H - 0.5)
    ego_f = ego_c.ravel().astype(np.float32)
    N = len(affs)
    out = np.empty(N, np.float32)
    for s0 in range(0, N, chunk):
        A = affs[s0 : s0 + chunk].astype(np.float32)
        n = len(A)
        ix = np.multiply.outer(A[:, 0, 0], gx)
        ix += np.multiply.outer(A[:, 0, 1], gy)
        ix += A[:, 0, 2, None]
        ix *= scx
        ix += ox
        iy = np.multiply.outer(A[:, 1, 0], gx)
        iy += np.multiply.outer(A[:, 1, 1], gy)
        iy += A[:, 1, 2, None]
        iy *= scy
        iy += oy
        x0 = np.floor(ix)
        y0 = np.floor(iy)
        wx = ix - x0
        wy = iy - y0
        xi = x0.astype(np.int32)
        xi += padb
        np.clip(xi, 0, Wp - 2, out=xi)
        yi = y0.astype(np.int32)
        yi += padb
        np.clip(yi, 0, Wp - 2, out=yi)
        base = yi
        base *= Wp
        base += xi
        b00 = flat[base]
        b01 = flat[base + 1]
        b10 = flat[base + Wp]
        b11 = flat[base + Wp + 1]
        top = (1.0 - wx) * b00
        top += wx * b01
        bot = (1.0 - wx) * b10
        bot += wx * b11
        val = (1.0 - wy) * top
        val += wy * bot
        out[s0 : s0 + n] = val @ ego_f
    return out


def _theta_warp_fields(base_2x3, dr, align_corners):
    """Pixel-coord sample positions of the theta-only warp (dx=dy=0)."""
    th = np.float32(dr) * np.float32(math.pi / 180.0)
    c, s = np.cos(th, dtype=np.float32), np.sin(th, dtype=np.float32)
    delta = np.array([[c, -s, 0], [s, c, 0], [0, 0, 1]], np.float32)
    base3 = np.concatenate([base_2x3, [[0, 0, 1]]], 0).astype(np.float32)
    aff = (base3 @ delta)[:2]
    xs, ys = _grid_1d(align_corners)
    gx = aff[0, 0] * xs[None, :] + aff[0, 1] * ys[:, None] + aff[0, 2]
    gy = aff[1, 0] * xs[None, :] + aff[1, 1] * ys[:, None] + aff[1, 2]
    if align_corners:
        ix = (gx + 1.0) * (0.5 * (W - 1))
        iy = (gy + 1.0) * (0.5 * (H - 1))
    else:
        ix = gx * (0.5 * W) + (0.5 * W - 0.5)
        iy = gy * (0.5 * H) + (0.5 * H - 0.5)
    return ix.astype(np.float32), iy.astype(np.float32)


def _trans_shifts(base_2x3, cands, align_corners):
    """Pixel-space shifts (ux, uy) each candidate translation adds."""
    B2 = base_2x3[:2, :2].astype(np.float32)
    tx = (2.0 * cands[:, 0] / (W - 1)).astype(np.float32)
    ty = (2.0 * cands[:, 1] / (H - 1)).astype(np.float32)
    if align_corners:
        sx, sy = 0.5 * (W - 1), 0.5 * (H - 1)
    else:
        sx, sy = 0.5 * W, 0.5 * H
    ux = (B2[0, 0] * tx + B2[0, 1] * ty) * np.float32(sx)
    uy = (B2[1, 0] * tx + B2[1, 1] * ty) * np.float32(sy)
    return ux, uy


def _build_splats(ego_c, ix, iy):
    """Moment splat canvases [4, CANVAS, CANVAS] f32, or None if out of range."""
    Xi = np.floor(ix)
    Yi = np.floor(iy)
    Xf = (ix - Xi).astype(np.float32)
    Yf = (iy - Yi).astype(np.float32)
    Xi = Xi.astype(np.int64)
    Yi = Yi.astype(np.int64)
    if (
        Xi.min() < -OFF
        or Xi.max() >= CANVAS - OFF
        or Yi.min() < -OFF
        or Yi.max() >= CANVAS - OFF
    ):
        return None
    S = np.zeros((M, CANVAS, CANVAS), np.float32)
    flatidx = ((Yi + OFF) * CANVAS + (Xi + OFF)).ravel()
    nbins = CANVAS * CANVAS
    for m, mu in enumerate((None, Xf, Yf, Xf * Yf)):
        wgt = ego_c if mu is None else mu * ego_c
        S[m] = (
            np.bincount(flatidx, weights=wgt.ravel().astype(np.float64), minlength=nbins)
            .reshape(CANVAS, CANVAS)
            .astype(np.float32)
        )
    return S


def _assemble_approx(T, base_2x3, cands, align_corners):
    """Approx scores for one theta's candidates from its surface T [NL, M, NL].

    Returns None if any candidate's lag falls outside the computed window
    (caller falls back to the exact host path)."""
    ux, uy = _trans_shifts(base_2x3, cands, align_corners)
    Ui = np.floor(ux).astype(np.int64)
    Ufx = (ux - Ui).astype(np.float32)
    Vi = np.floor(uy).astype(np.int64)
    Ufy = (uy - Vi).astype(np.float32)
    if (
        Ui.min() < LMIN
        or Ui.max() + 1 >= LMIN + NL
        or Vi.min() < LMIN
        or Vi.max() + 1 >= LMIN + NL
    ):
        return None
    out = np.zeros(len(cands), np.float32)
    for j in (0, 1):
        ay = np.where(j, Ufy, 1.0 - Ufy).astype(np.float32)
        by = 1.0 if j else -1.0
        Jp = Vi + j - LMIN
        for k in (0, 1):
            ax = np.where(k, Ufx, 1.0 - Ufx).astype(np.float32)
            bx = 1.0 if k else -1.0
            Kp = Ui + k - LMIN
            out += ax * ay * T[Kp, 0, Jp]
            out += bx * ay * T[Kp, 1, Jp]
            out += ax * by * T[Kp, 2, Jp]
            out += bx * by * T[Kp, 3, Jp]
    return out


def _build_nrq(nbr_c):
    """Host-side repacked window tensor [KI, 2, NI, MMQ] fp32.

    nrq[ki, half, i, 2t+slot] = nbr_c[2i+slot, (21 + ki + 88*half) + t - 68]
    with zero padding outside [0, W)."""
    # padded transposed nbr: P[xp, y] = nbr[y, xp - 47], xp in [0, 224)
    P = np.zeros((KI + 88 + NL - 1 + 1, H), np.float32)  # 224 x 128
    P[47 : 47 + W, :] = nbr_c.T
    s0, s1 = P.strides
    A = np.lib.stride_tricks.as_strided(
        P, shape=(KI, 2, NL, H), strides=(s0, 88 * s0, s0, s1)
    )
    # [ki, half, t, y] -> [ki, half, i, 2t+slot]
    A5 = A.reshape(KI, 2, NL, NI, 2)           # [ki, half, t, i, slot]
    out = np.ascontiguousarray(A5.transpose(0, 1, 3, 2, 4)).reshape(KI, 2, NI, MMQ)
    return out


def _build_st(S_list):
    """Device splat tensor [KI, 2, G, RP, PF] fp32 from 4 splat canvases.

    st[ki, half, g, r', 4*tp + m] = S_{2g+tp}[m, ROWBASE - r', 21 + ki + 88*half]
    """
    st = np.zeros((KI, 2, G, RP, PF), np.float32)
    for th in range(U):
        g, tp = divmod(th, 2)
        S = S_list[th]
        for m in range(M):
            # rows ROWBASE..ROWBASE-RP+1, cols CL0..CL0+176
            Srev = S[m, ROWBASE : ROWBASE - RP : -1, CL0 : CL0 + 2 * KI]  # [RP, 176]
            cv = Srev.T.reshape(2, KI, RP)  # [half, ki, r']
            st[:, :, g, :, 4 * tp + m] = cv.transpose(1, 0, 2)
    return st


def _fold_tout(tout):
    """Device output [G*MMQ, NFREE] f32 -> per-theta surfaces [U, NL, M, NL]."""
    t5 = tout.reshape(G, MMQ, NJ, 2, M)  # [g, 2t+slot, j, tp, m]
    out = np.empty((U, NL, M, NL), np.float32)
    for th in range(U):
        g, tp = divmod(th, 2)
        a0 = t5[g, 0::2, :, tp, :]  # [t, j, m]
        a1 = t5[g, 1::2, :, tp, :]
        Tf = a0[:, 1 : 1 + NL, :] + a1[:, 0:NL, :]  # [t, J, m]
        out[th] = Tf.transpose(0, 2, 1)  # [K, m, J]
    return out


# ----------------------------------------------------------------------------
# device program
# ----------------------------------------------------------------------------

def _get_nc():
    global _NC
    if _NC is not None:
        return _NC
    sys.path.insert(0, "/opt/trn_rl_repo")
    from contextlib import ExitStack

    import concourse.bass as bass
    import concourse.mybir as mybir
    import concourse.tile as tile
    from concourse import bacc

    fp8 = mybir.dt.float8e4
    nc = bacc.Bacc("TRN2", target_bir_lowering=False, debug=False)
    nrq = nc.declare_dram_parameter("nrq", [KI, 2 * NI * MMQ], fp8, isOutput=False)
    st = nc.declare_dram_parameter("st", [KI, 2 * G * RP * PF], fp8, isOutput=False)
    tout = nc.declare_dram_parameter(
        "tout", [G * MMQ, NFREE], mybir.dt.float32, isOutput=True
    )
    nrq_h = nrq.tensor if isinstance(nrq, bass.AP) else nrq
    st_h = st.tensor if isinstance(st, bass.AP) else st
    tout_h = tout.tensor if isinstance(tout, bass.AP) else tout

    DR = mybir.MatmulPerfMode.DoubleRow
    with ExitStack() as ctx:
        tc = ctx.enter_context(tile.TileContext(nc))
        pool = ctx.enter_context(tc.tile_pool(name="persist", bufs=1))
        psum_pool = ctx.enter_context(tc.tile_pool(name="psum", bufs=2, space="PSUM"))
        stage_pool = ctx.enter_context(tc.tile_pool(name="stage", bufs=2))

        nrq_t = pool.tile([KI, 2 * NI * MMQ], fp8)
        st_t = pool.tile([KI, 2 * G * RP * PF], fp8)
        warm_sb = pool.tile([128, 512], fp8)

        # PE warm-up: dummy matmuls with no DMA dependency keep the PE busy
        # during the input DMA so the HAM clock-gate opens before the real
        # accumulation starts.
        warm_ps = psum_pool.tile([128, 512], mybir.dt.float32, name="warm", tag="warm")
        nc.vector.memset(warm_sb[:], 0)
        for _ in range(9):
            nc.tensor.matmul(
                warm_ps[:], warm_sb[:, 0:128], warm_sb[:], start=True, stop=True
            )

        nc.sync.dma_start(out=nrq_t[:], in_=nrq_h[:])
        nc.sync.dma_start(out=st_t[:], in_=st_h[:])

        psums = [
            psum_pool.tile([MMQ, NFREE], mybir.dt.float32, name=f"ps{g}", tag=f"ps{g}")
            for g in range(G)
        ]
        wb = nrq_t[:]
        sb = st_t[:]
        HW_NRQ = 2 * NI * MMQ // 2  # 6144: half stride in nrq tile
        HW_ST = G * RP * PF         # 2816: half stride in st tile
        for i in range(NI):
            Wap = bass.AP(
                tensor=wb.tensor,
                offset=wb.offset + i * MMQ,
                ap=[list(wb.ap[0]), [HW_NRQ, 2], [1, MMQ]],
            )
            roff = (R0 - 2 * i) * PF
            for g in range(G):
                Xap = bass.AP(
                    tensor=sb.tensor,
                    offset=sb.offset + g * (RP * PF) + roff,
                    ap=[list(sb.ap[0]), [HW_ST, 2], [1, NFREE]],
                )
                nc.tensor.matmul(
                    psums[g][:],
                    Wap,
                    Xap,
                    start=(i == 0),
                    stop=(i == NI - 1),
                    perf_mode=DR,
                )

        for g in range(G):
            stg = stage_pool.tile(
                [MMQ, NFREE], mybir.dt.float32, name=f"stg{g}", tag=f"stg{g}"
            )
            nc.scalar.copy(stg[:], psums[g][:])
            dst = bass.AP(
                tensor=tout_h,
                offset=g * MMQ * NFREE,
                ap=[[NFREE, MMQ], [1, NFREE]],
            )
            nc.sync.dma_start(out=dst, in_=stg[:])
    nc.compile()
    _NC = nc
    return nc


def _run_device(in_maps):
    sys.path.insert(0, "/opt/trn_rl_repo")
    import ml_dtypes
    from concourse.bass_utils import run_bass_kernel_spmd

    maps = [
        {
            "nrq": np.ascontiguousarray(m["nrq"].reshape(KI, -1)).astype(
                ml_dtypes.float8_e4m3
            ),
            "st": np.ascontiguousarray(m["st"].reshape(KI, -1)).astype(
                ml_dtypes.float8_e4m3
            ),
        }
        for m in in_maps
    ]
    global LAST_MAPS, LAST_TOUTS
    LAST_MAPS = maps
    res = run_bass_kernel_spmd(_get_nc(), maps, core_ids=list(range(len(maps))))
    LAST_TOUTS = [r["tout"].astype(np.float32) for r in res.results]
    return [_fold_tout(t) for t in LAST_TOUTS]


# ----------------------------------------------------------------------------
# pipeline
# ----------------------------------------------------------------------------

def _refine_pair_host_only(ego_c, nbr_c, base, align_corners):
    """Pure-host exact fallback (pathological inputs only)."""
    nbrP = _pad_nbr(nbr_c)
    cands = _coarse_cands()
    sc = _exact_scores(ego_c, nbrP, _cand_affines(cands, base), align_corners)
    bi = int(np.argmax(sc))
    cp = cands[bi] if sc[bi] > 1e-5 else np.zeros(3, np.float32)
    if np.all(cp == 0.0):
        return base
    fc = _fine_cands(cp)
    affs_f = _cand_affines(fc, base)
    sf = _exact_scores(ego_c, nbrP, affs_f, align_corners)
    bif = int(np.argmax(sf))
    return affs_f[bif] if sf[bif] > 1e-5 else base


def _finish_pair(ego_c, nbrP, base, cands, approx, align_corners):
    """Adaptive exact rescore of the approx-selected coarse set -> cp."""
    thresh = approx.max() - DELTA_COARSE
    sel = np.where(approx >= thresh)[0]
    if len(sel) > RESCORE_CAP:
        sel = sel[np.argsort(approx[sel])[::-1][:RESCORE_CAP]]
    if len(sel) < 48:
        sel = np.argsort(approx)[::-1][:48]
    affs = _cand_affines(cands[sel], base)
    sc = _exact_scores(ego_c, nbrP, affs, align_corners)
    bi_local = int(np.argmax(sc))
    bi = int(sel[bi_local])
    ok = sc[bi_local] > 1e-5
    cp = cands[bi] if ok else np.zeros(3, np.float32)
    return cp


def kernel(occ_map, record_len, affine_matrix, align_corners):
    occ = np.asarray(occ_map, dtype=np.float32)
    rl = np.asarray(record_len).reshape(-1)
    aff_in = np.asarray(affine_matrix)
    out_dtype = aff_in.dtype
    refined = aff_in.astype(np.float32).copy()
    ac = bool(np.asarray(align_corners))

    # pair list exactly as the reference builds it
    pairs = []
    idx = 0
    for b in range(len(rl)):
        n_agents = int(rl[b])
        grp0 = idx
        idx += n_agents
        if n_agents <= 1:
            continue
        for n in range(1, n_agents):
            pairs.append((b, n, grp0, grp0 + n))
    if not pairs:
        return refined.astype(out_dtype)

    device_ok = (
        len(pairs) <= 2
        and all(
            b < refined.shape[0] and n < refined.shape[2] and nb < occ.shape[0]
            for (b, n, _, nb) in pairs
        )
    )

    pair_data = []
    for (b, n, ei, ni) in pairs:
        # mimic jax OOB semantics: clip gather indices, drop OOB scatters
        ei = min(ei, occ.shape[0] - 1)
        ni = min(ni, occ.shape[0] - 1)
        ego = occ[ei, 0]
        nbr = occ[ni, 0]
        ego_c = np.where(ego > THRESH, ego, 0.0).astype(np.float32)
        nbr_c = np.where(nbr > THRESH, nbr, 0.0).astype(np.float32)
        base = refined[b, 0, n].astype(np.float32)
        pair_data.append(
            {
                "b": min(b, refined.shape[0] - 1),
                "n": n,
                "ego_c": ego_c,
                "nbr_c": nbr_c,
                "nbrP": _pad_nbr(nbr_c),
                "base": base,
            }
        )

    cands = _coarse_cands()
    drs = np.unique(cands[:, 2])  # 16 rotations
    by_dr = {float(dr): np.where(cands[:, 2] == dr)[0] for dr in drs}

    # build device inputs: 16 theta-units per pair, 4 per core; cores 0-3
    # pair0, cores 4-7 pair1
    use_device = device_ok
    unit_map = {}  # (core, slot) -> (pair_idx, dr)
    in_maps = None
    if use_device:
        zero_nrq = np.zeros((KI, 2, NI, MMQ), np.float32)
        zero_st = np.zeros((KI, 2, G, RP, PF), np.float32)
        in_maps = []
        splat_fail = False
        nrq_cache = {}
        for core in range(N_CORES):
            pi = core // 4
            if pi >= len(pair_data):
                in_maps.append({"nrq": zero_nrq, "st": zero_st})
                continue
            pd = pair_data[pi]
            S_list = []
            for slot in range(U):
                th_idx = 4 * (core % 4) + slot
                dr = float(drs[th_idx])
                ix, iy = _theta_warp_fields(pd["base"], dr, ac)
                S = _build_splats(pd["ego_c"], ix, iy)
                if S is None:
                    splat_fail = True
                    break
                S_list.append(S)
                unit_map[(core, slot)] = (pi, dr)
            if splat_fail:
                break
            if pi not in nrq_cache:
                nrq_cache[pi] = _build_nrq(pd["nbr_c"])
            in_maps.append({"nrq": nrq_cache[pi], "st": _build_st(S_list)})
        if splat_fail:
            use_device = False

    if use_device:
        try:
            touts = _run_device(in_maps)
        except Exception:
            use_device = False

    for pi, pd in enumerate(pair_data):
        base = pd["base"]
        pair_device = use_device
        approx = None
        if pair_device:
            approx = np.empty(len(cands), np.float32)
            for core in range(4 * pi, 4 * pi + 4):
                for slot in range(U):
                    key = (core, slot)
                    if key not in unit_map:
                        continue
                    _, dr = unit_map[key]
                    sel = by_dr[dr]
                    a = _assemble_approx(touts[core][slot], base, cands[sel], ac)
                    if a is None:
                        pair_device = False
                        break
                    approx[sel] = a
                if not pair_device:
                    break
        if pair_device:
            cp = _finish_pair(pd["ego_c"], pd["nbrP"], base, cands, approx, ac)
            if np.all(cp == 0.0):
                new_aff = base
            else:
                fc = _fine_cands(cp)
                affs_f = _cand_affines(fc, base)
                sf = _exact_scores(pd["ego_c"], pd["nbrP"], affs_f, ac)
                bif = int(np.argmax(sf))
                new_aff = affs_f[bif] if sf[bif] > 1e-5 else base
        else:
            new_aff = _refine_pair_host_only(pd["ego_c"], pd["nbr_c"], base, ac)
        if pd["n"] < refined.shape[2] and pd["b"] < refined.shape[0]:
            refined[pd["b"], 0, pd["n"]] = new_aff

    return refined.astype(out_dtype)


# revision 10
# speedup vs baseline: 1.2628x; 1.2628x over previous
"""Trainium2 Bass kernel for nn_CartographerPoseCorrector (v2: fp8 DoubleRow).

Same algorithm as v1 (moment-correlation surfaces on TensorE + exact host
rescore of a margin set), restructured for speed:

- fp8e4m3 inputs + DoubleRow matmuls: contraction pairs the two halves of
  the 176-column canvas window, so one matmul contracts all columns.
- Lag windows trimmed to the candidate range actually reachable
  (48 x 48 lags, J-window 49).
- The (theta', m) axes are interleaved *inside* the canvas-row axis of the
  splat tensor, so each accumulation step's J-window is one contiguous
  392-element slice -> a legal 3-dim DoubleRow rhs AP.
- The nbr window tensor arrives from the host already repacked
  (no on-chip DVE repack).
- Dummy warm-up matmuls run during the input DMA to lift the PE HAM
  clock-gate before the real accumulation starts.

Geometry (device):
  W[ki, half, i, 2t+slot] = nbr[2i+slot, c + t - 68],  c = 21 + ki + 88*half
  X[ki, half, g, r', 4*tp+m] = S_{2g+tp}[m, 195 - r', c]
  psum[g][2t+slot, j*8 + tp*4 + m]
      += sum_{ki,half,i} W[ki,half,i,2t+slot] * X[ki,half,g,126-2i+j, ...]
  => T_m,theta[K=t-24, J=j+slot-25] after the host fold.
"""

import math
import sys

import numpy as np

H = W = 128
THRESH = 0.3
TRANS_RANGE = 20.0
ROT_RANGE = 15.0
COARSE_STEP = 2.0
FINE_STEP = 0.5

# Device-kernel geometry (must match the Bass program)
CANVAS = 224     # splat canvas extent (host-side bounds check)
OFF = 44         # image coord -> canvas coord offset
NL = 48          # lags per axis
LMIN = -24       # lag range [LMIN, LMIN + NL)
NJ = NL + 1      # J-window width per slot
MMQ = 2 * NL     # weight columns (= psum partitions)
KI = 88          # contraction partitions (x2 halves = 176 canvas cols)
CL0 = 21         # first canvas column on device
XOFF2 = 68       # nbr col = c + t - XOFF2
R0 = 126         # window offset: roff = R0 - 2*i
ROWBASE = 195    # S row = ROWBASE - r'
RP = 176         # r' extent
G = 2            # theta groups per core (2 thetas each)
PF = 8           # (theta', m) interleave factor inside r'
NI = H // 2      # accumulation steps
M = 4            # moments
U = 4            # thetas per core
N_CORES = 8
NFREE = NJ * PF  # 392: psum free size

DELTA_COARSE = 320.0   # exact-rescore safety margin
RESCORE_CAP = 2800     # hard cap on rescored coarse candidates per pair

_NC = None
LAST_MAPS = None  # debug/timing aid: last device input maps (fp8-converted)
LAST_TOUTS = None  # debug aid: last raw device outputs


# ----------------------------------------------------------------------------
# host math (mirrors reference numerics in fp32 where it matters)
# ----------------------------------------------------------------------------

def _grid_1d(align_corners):
    if align_corners:
        xs = np.linspace(-1.0, 1.0, W, dtype=np.float32)
        ys = np.linspace(-1.0, 1.0, H, dtype=np.float32)
    else:
        xs = ((2.0 * np.arange(W, dtype=np.float32) + 1.0) / W - 1.0)
        ys = ((2.0 * np.arange(H, dtype=np.float32) + 1.0) / H - 1.0)
    return xs, ys


def _coarse_cands():
    dxs = np.arange(-TRANS_RANGE, TRANS_RANGE + 1e-3, COARSE_STEP, dtype=np.float32)
    drs = np.arange(-ROT_RANGE, ROT_RANGE + 1e-3, COARSE_STEP, dtype=np.float32)
    gdx, gdy, gdr = np.meshgrid(dxs, dxs, drs, indexing="ij")
    return np.stack([gdx.ravel(), gdy.ravel(), gdr.ravel()], axis=1)


def _fine_cands(cp):
    off = np.arange(-COARSE_STEP, COARSE_STEP + 1e-3, FINE_STEP, dtype=np.float32)
    gdx, gdy, gdr = np.meshgrid(cp[0] + off, cp[1] + off, cp[2] + off, indexing="ij")
    return np.stack([gdx.ravel(), gdy.ravel(), gdr.ravel()], axis=1)


def _cand_affines(cands, base_2x3):
    dx, dy, dr = cands[:, 0], cands[:, 1], cands[:, 2]
    tx = (2.0 * dx / max(W - 1, 1)).astype(np.float32)
    ty = (2.0 * dy / max(H - 1, 1)).astype(np.float32)
    th = (dr * np.float32(math.pi / 180.0)).astype(np.float32)
    c, s = np.cos(th), np.sin(th)
    z, o = np.zeros_like(c), np.ones_like(c)
    delta = np.stack([c, -s, tx, s, c, ty, z, z, o], axis=-1).reshape(-1, 3, 3)
    base3 = np.concatenate([base_2x3, np.array([[0, 0, 1]], np.float32)], axis=0)
    return np.einsum("ij,njk->nik", base3.astype(np.float32), delta.astype(np.float32))[
        :, :2, :
    ].astype(np.float32)


def _pad_nbr(nbr_c, padb=8):
    out = np.zeros((H + 2 * padb, W + 2 * padb), np.float32)
    out[padb : padb + H, padb : padb + W] = nbr_c
    return out


def _exact_scores(ego_c, nbrP, affs, align_corners, padb=8, chunk=16):
    """Exact fp32 bilinear grid-sample scores for candidate affines [n,2,3]."""
    xs, ys = _grid_1d(align_corners)
    gx = np.broadcast_to(xs[None, :], (H, W)).ravel().astype(np.float32)
    gy = np.broadcast_to(ys[:, None], (H, W)).ravel().astype(np.float32)
    flat = nbrP.ravel()
    Wp = nbrP.shape[1]
    if align_corners:
        scx, ox = np.float32(0.5 * (W - 1)), np.float32(0.5 * (W - 1))
        scy, oy = np.float32(0.5 * (H - 1)), np.float32(0.5 * (H - 1))
    else:
        scx, ox = np.float32(0.5 * W), np.float32(0.5 * W - 0.5)
        scy, oy = np.float32(0.5 * H), np.float32(0.5 * H - 0.5)
    ego_f = ego_c.ravel().astype(np.float32)
    N = len(affs)
    out = np.empty(N, np.float32)
    for s0 in range(0, N, chunk):
        A = affs[s0 : s0 + chunk].astype(np.float32)
        n = len(A)
        ix = np.multiply.outer(A[:, 0, 0], gx)
        ix += np.multiply.outer(A[:, 0, 1], gy)
        ix += A[:, 0, 2, None]
        ix *= scx
        ix += ox
        iy = np.multiply.outer(A[:, 1, 0], gx)
        iy += np.multiply.outer(A[:, 1, 1], gy)
        iy += A[:, 1, 2, None]
        iy *= scy
        iy += oy
        x0 = np.floor(ix)
        y0 = np.floor(iy)
        wx = ix - x0
        wy = iy - y0
        xi = x0.astype(np.int32)
        xi += padb
        np.clip(xi, 0, Wp - 2, out=xi)
        yi = y0.astype(np.int32)
        yi += padb
        np.clip(yi, 0, Wp - 2, out=yi)
        base = yi
        base *= Wp
        base += xi
        b00 = flat[base]
        b01 = flat[base + 1]
        b10 = flat[base + Wp]
        b11 = flat[base + Wp + 1]
        top = (1.0 - wx) * b00
        top += wx * b01
        bot = (1.0 - wx) * b10
        bot += wx * b11
        val = (1.0 - wy) * top
        val += wy * bot
        out[s0 : s0 + n] = val @ ego_f
    return out


def _theta_warp_fields(base_2x3, dr, align_corners):
    """Pixel-coord sample positions of the theta-only warp (dx=dy=0)."""
    th = np.float32(dr) * np.float32(math.pi / 180.0)
    c, s = np.cos(th, dtype=np.float32), np.sin(th, dtype=np.float32)
    delta = np.array([[c, -s, 0], [s, c, 0], [0, 0, 1]], np.float32)
    base3 = np.concatenate([base_2x3, [[0, 0, 1]]], 0).astype(np.float32)
    aff = (base3 @ delta)[:2]
    xs, ys = _grid_1d(align_corners)
    gx = aff[0, 0] * xs[None, :] + aff[0, 1] * ys[:, None] + aff[0, 2]
    gy = aff[1, 0] * xs[None, :] + aff[1, 1] * ys[:, None] + aff[1, 2]
    if align_corners:
        ix = (gx + 1.0) * (0.5 * (W - 1))
        iy = (gy + 1.0) * (0.5 * (H - 1))
    else:
        ix = gx * (0.5 * W) + (0.5 * W - 0.5)
        iy = gy * (0.5 * H) + (0.5 * H - 0.5)
    return ix.astype(np.float32), iy.astype(np.float32)


def _trans_shifts(base_2x3, cands, align_corners):
    """Pixel-space shifts (ux, uy) each candidate translation adds."""
    B2 = base_2x3[:2, :2].astype(np.float32)
    tx = (2.0 * cands[:, 0] / (W - 1)).astype(np.float32)
    ty = (2.0 * cands[:, 1] / (H - 1)).astype(np.float32)
    if align_corners:
        sx, sy = 0.5 * (W - 1), 0.5 * (H - 1)
    else:
        sx, sy = 0.5 * W, 0.5 * H
    ux = (B2[0, 0] * tx + B2[0, 1] * ty) * np.float32(sx)
    uy = (B2[1, 0] * tx + B2[1, 1] * ty) * np.float32(sy)
    return ux, uy


def _build_splats(ego_c, ix, iy):
    """Moment splat canvases [4, CANVAS, CANVAS] f32, or None if out of range."""
    Xi = np.floor(ix)
    Yi = np.floor(iy)
    Xf = (ix - Xi).astype(np.float32)
    Yf = (iy - Yi).astype(np.float32)
    Xi = Xi.astype(np.int64)
    Yi = Yi.astype(np.int64)
    if (
        Xi.min() < -OFF
        or Xi.max() >= CANVAS - OFF
        or Yi.min() < -OFF
        or Yi.max() >= CANVAS - OFF
    ):
        return None
    S = np.zeros((M, CANVAS, CANVAS), np.float32)
    flatidx = ((Yi + OFF) * CANVAS + (Xi + OFF)).ravel()
    nbins = CANVAS * CANVAS
    for m, mu in enumerate((None, Xf, Yf, Xf * Yf)):
        wgt = ego_c if mu is None else mu * ego_c
        S[m] = (
            np.bincount(flatidx, weights=wgt.ravel().astype(np.float64), minlength=nbins)
            .reshape(CANVAS, CANVAS)
            .astype(np.float32)
        )
    return S


def _assemble_approx(T, base_2x3, cands, align_corners):
    """Approx scores for one theta's candidates from its surface T [NL, M, NL].

    Returns None if any candidate's lag falls outside the computed window
    (caller falls back to the exact host path)."""
    ux, uy = _trans_shifts(base_2x3, cands, align_corners)
    Ui = np.floor(ux).astype(np.int64)
    Ufx = (ux - Ui).astype(np.float32)
    Vi = np.floor(uy).astype(np.int64)
    Ufy = (uy - Vi).astype(np.float32)
    if (
        Ui.min() < LMIN
        or Ui.max() + 1 >= LMIN + NL
        or Vi.min() < LMIN
        or Vi.max() + 1 >= LMIN + NL
    ):
        return None
    out = np.zeros(len(cands), np.float32)
    for j in (0, 1):
        ay = np.where(j, Ufy, 1.0 - Ufy).astype(np.float32)
        by = 1.0 if j else -1.0
        Jp = Vi + j - LMIN
        for k in (0, 1):
            ax = np.where(k, Ufx, 1.0 - Ufx).astype(np.float32)
            bx = 1.0 if k else -1.0
            Kp = Ui + k - LMIN
            out += ax * ay * T[Kp, 0, Jp]
            out += bx * ay * T[Kp, 1, Jp]
            out += ax * by * T[Kp, 2, Jp]
            out += bx * by * T[Kp, 3, Jp]
    return out


def _build_nrq(nbr_c):
    """Host-side repacked window tensor [KI, NI, 2, MMQ] fp32.

    nrq[ki, i, half, 2t+slot] = nbr_c[2i+slot, (21 + ki + 88*half) + t - 68]
    with zero padding outside [0, W).  (i-major so the device can DMA it in
    independent i-chunks; half is the DoubleRow pair dim.)"""
    # padded transposed nbr: P[xp, y] = nbr[y, xp - 47], xp in [0, 224)
    P = np.zeros((KI + 88 + NL - 1 + 1, H), np.float32)  # 224 x 128
    P[47 : 47 + W, :] = nbr_c.T
    s0, s1 = P.strides
    A = np.lib.stride_tricks.as_strided(
        P, shape=(KI, 2, NL, H), strides=(s0, 88 * s0, s0, s1)
    )
    # [ki, half, t, y] -> [ki, i, half, 2t+slot]
    A5 = A.reshape(KI, 2, NL, NI, 2)           # [ki, half, t, i, slot]
    out = np.ascontiguousarray(A5.transpose(0, 3, 1, 2, 4)).reshape(KI, NI, 2, MMQ)
    return out


def _build_st(S_list):
    """Device splat tensor [KI, 2, G, RP, PF] fp32 from 4 splat canvases.

    st[ki, half, g, r', 4*tp + m] = S_{2g+tp}[m, ROWBASE - r', 21 + ki + 88*half]
    """
    st = np.zeros((KI, 2, G, RP, PF), np.float32)
    for th in range(U):
        g, tp = divmod(th, 2)
        S = S_list[th]
        for m in range(M):
            # rows ROWBASE..ROWBASE-RP+1, cols CL0..CL0+176
            Srev = S[m, ROWBASE : ROWBASE - RP : -1, CL0 : CL0 + 2 * KI]  # [RP, 176]
            cv = Srev.T.reshape(2, KI, RP)  # [half, ki, r']
            st[:, :, g, :, 4 * tp + m] = cv.transpose(1, 0, 2)
    return st


def _fold_tout(tout):
    """Device output [G*MMQ, NFREE] f32 -> per-theta surfaces [U, NL, M, NL]."""
    t5 = tout.reshape(G, MMQ, NJ, 2, M)  # [g, 2t+slot, j, tp, m]
    out = np.empty((U, NL, M, NL), np.float32)
    for th in range(U):
        g, tp = divmod(th, 2)
        a0 = t5[g, 0::2, :, tp, :]  # [t, j, m]
        a1 = t5[g, 1::2, :, tp, :]
        Tf = a0[:, 1 : 1 + NL, :] + a1[:, 0:NL, :]  # [t, J, m]
        out[th] = Tf.transpose(0, 2, 1)  # [K, m, J]
    return out


# ----------------------------------------------------------------------------
# device program
# ----------------------------------------------------------------------------

def _get_nc():
    global _NC
    if _NC is not None:
        return _NC
    sys.path.insert(0, "/opt/trn_rl_repo")
    from contextlib import ExitStack

    import concourse.bass as bass
    import concourse.mybir as mybir
    import concourse.tile as tile
    from concourse import bacc

    fp8 = mybir.dt.float8e4
    nc = bacc.Bacc("TRN2", target_bir_lowering=False, debug=False)
    nrq = nc.declare_dram_parameter("nrq", [KI, NI * 2 * MMQ], fp8, isOutput=False)
    st = nc.declare_dram_parameter("st", [KI, 2 * G * RP * PF], fp8, isOutput=False)
    tout = nc.declare_dram_parameter(
        "tout", [G * MMQ, NFREE], mybir.dt.float32, isOutput=True
    )
    nrq_h = nrq.tensor if isinstance(nrq, bass.AP) else nrq
    st_h = st.tensor if isinstance(st, bass.AP) else st
    tout_h = tout.tensor if isinstance(tout, bass.AP) else tout

    DR = mybir.MatmulPerfMode.DoubleRow
    NCHUNK = 4
    CI = NI // NCHUNK            # 16 i-steps per nrq chunk
    CB = CI * 2 * MMQ            # 3072 B per partition per chunk
    with ExitStack() as ctx:
        tc = ctx.enter_context(tile.TileContext(nc))
        pool = ctx.enter_context(tc.tile_pool(name="persist", bufs=1))
        psum_pool = ctx.enter_context(tc.tile_pool(name="psum", bufs=2, space="PSUM"))

        st_t = pool.tile([KI, 2 * G * RP * PF], fp8)
        nrq_c = [pool.tile([KI, CB], fp8, name=f"nrqc{k}", tag=f"nrqc{k}") for k in range(NCHUNK)]
        warm_sb = pool.tile([128, 512], fp8)

        # PE warm-up: dummy matmuls with no DMA dependency keep the PE busy
        # during the input DMA so the HAM clock-gate opens before the real
        # accumulation starts.
        warm_ps = psum_pool.tile([128, 512], mybir.dt.float32, name="warm", tag="warm")
        nc.vector.memset(warm_sb[:], 0)
        for _ in range(14):
            nc.tensor.matmul(
                warm_ps[:], warm_sb[:, 0:128], warm_sb[:], start=True, stop=True
            )

        # inputs spread over the three DMA paths (sync/scalar HWDGE queues,
        # gpsimd SWDGE) so the per-queue per-partition packet streams overlap
        nc.sync.dma_start(
            out=st_t[:],
            in_=bass.AP(tensor=st_h, offset=0,
                        ap=[[2 * G * RP * PF, KI], [1, 2 * G * RP * PF]]),
        )
        for k in range(NCHUNK):
            eng = nc.scalar if k < 2 else nc.gpsimd
            eng.dma_start(
                out=nrq_c[k][:],
                in_=bass.AP(tensor=nrq_h, offset=k * CB,
                            ap=[[NI * 2 * MMQ, KI], [1, CB]]),
            )

        psums = [
            psum_pool.tile([MMQ, NFREE], mybir.dt.float32, name=f"ps{g}", tag=f"ps{g}")
            for g in range(G)
        ]
        sb = st_t[:]
        HW_ST = G * RP * PF         # 2816: half stride in st tile
        for i in range(NI):
            wb = nrq_c[i // CI][:]
            Wap = bass.AP(
                tensor=wb.tensor,
                offset=wb.offset + (i % CI) * (2 * MMQ),
                ap=[list(wb.ap[0]), [MMQ, 2], [1, MMQ]],
            )
            roff = (R0 - 2 * i) * PF
            for g in range(G):
                Xap = bass.AP(
                    tensor=sb.tensor,
                    offset=sb.offset + g * (RP * PF) + roff,
                    ap=[list(sb.ap[0]), [HW_ST, 2], [1, NFREE]],
                )
                nc.tensor.matmul(
                    psums[g][:],
                    Wap,
                    Xap,
                    start=(i == 0),
                    stop=(i == NI - 1),
                    perf_mode=DR,
                )

        stage_pool = ctx.enter_context(tc.tile_pool(name="stage", bufs=2))
        for g in range(G):
            stg = stage_pool.tile(
                [MMQ, NFREE], mybir.dt.float32, name=f"stg{g}", tag=f"stg{g}"
            )
            nc.vector.tensor_copy(stg[:], psums[g][:])
            dst = bass.AP(
                tensor=tout_h,
                offset=g * MMQ * NFREE,
                ap=[[NFREE, MMQ], [1, NFREE]],
            )
            nc.sync.dma_start(out=dst, in_=stg[:])
    nc.compile()
    _NC = nc
    return nc


def _run_device(in_maps):
    sys.path.insert(0, "/opt/trn_rl_repo")
    import ml_dtypes
    from concourse.bass_utils import run_bass_kernel_spmd

    maps = [
        {
            "nrq": np.ascontiguousarray(m["nrq"].reshape(KI, -1)).astype(
                ml_dtypes.float8_e4m3
            ),
            "st": np.ascontiguousarray(m["st"].reshape(KI, -1)).astype(
                ml_dtypes.float8_e4m3
            ),
        }
        for m in in_maps
    ]
    global LAST_MAPS, LAST_TOUTS
    LAST_MAPS = maps
    res = run_bass_kernel_spmd(_get_nc(), maps, core_ids=list(range(len(maps))))
    LAST_TOUTS = [r["tout"].astype(np.float32) for r in res.results]
    return [_fold_tout(t) for t in LAST_TOUTS]


# ----------------------------------------------------------------------------
# pipeline
# ----------------------------------------------------------------------------

def _refine_pair_host_only(ego_c, nbr_c, base, align_corners):
    """Pure-host exact fallback (pathological inputs only)."""
    nbrP = _pad_nbr(nbr_c)
    cands = _coarse_cands()
    sc = _exact_scores(ego_c, nbrP, _cand_affines(cands, base), align_corners)
    bi = int(np.argmax(sc))
    cp = cands[bi] if sc[bi] > 1e-5 else np.zeros(3, np.float32)
    if np.all(cp == 0.0):
        return base
    fc = _fine_cands(cp)
    affs_f = _cand_affines(fc, base)
    sf = _exact_scores(ego_c, nbrP, affs_f, align_corners)
    bif = int(np.argmax(sf))
    return affs_f[bif] if sf[bif] > 1e-5 else base


def _finish_pair(ego_c, nbrP, base, cands, approx, align_corners):
    """Adaptive exact rescore of the approx-selected coarse set -> cp."""
    thresh = approx.max() - DELTA_COARSE
    sel = np.where(approx >= thresh)[0]
    if len(sel) > RESCORE_CAP:
        sel = sel[np.argsort(approx[sel])[::-1][:RESCORE_CAP]]
    if len(sel) < 48:
        sel = np.argsort(approx)[::-1][:48]
    affs = _cand_affines(cands[sel], base)
    sc = _exact_scores(ego_c, nbrP, affs, align_corners)
    bi_local = int(np.argmax(sc))
    bi = int(sel[bi_local])
    ok = sc[bi_local] > 1e-5
    cp = cands[bi] if ok else np.zeros(3, np.float32)
    return cp


def kernel(occ_map, record_len, affine_matrix, align_corners):
    occ = np.asarray(occ_map, dtype=np.float32)
    rl = np.asarray(record_len).reshape(-1)
    aff_in = np.asarray(affine_matrix)
    out_dtype = aff_in.dtype
    refined = aff_in.astype(np.float32).copy()
    ac = bool(np.asarray(align_corners))

    # pair list exactly as the reference builds it
    pairs = []
    idx = 0
    for b in range(len(rl)):
        n_agents = int(rl[b])
        grp0 = idx
        idx += n_agents
        if n_agents <= 1:
            continue
        for n in range(1, n_agents):
            pairs.append((b, n, grp0, grp0 + n))
    if not pairs:
        return refined.astype(out_dtype)

    device_ok = (
        len(pairs) <= 2
        and all(
            b < refined.shape[0] and n < refined.shape[2] and nb < occ.shape[0]
            for (b, n, _, nb) in pairs
        )
    )

    pair_data = []
    for (b, n, ei, ni) in pairs:
        # mimic jax OOB semantics: clip gather indices, drop OOB scatters
        ei = min(ei, occ.shape[0] - 1)
        ni = min(ni, occ.shape[0] - 1)
        ego = occ[ei, 0]
        nbr = occ[ni, 0]
        ego_c = np.where(ego > THRESH, ego, 0.0).astype(np.float32)
        nbr_c = np.where(nbr > THRESH, nbr, 0.0).astype(np.float32)
        base = refined[b, 0, n].astype(np.float32)
        pair_data.append(
            {
                "b": min(b, refined.shape[0] - 1),
                "n": n,
                "ego_c": ego_c,
                "nbr_c": nbr_c,
                "nbrP": _pad_nbr(nbr_c),
                "base": base,
            }
        )

    cands = _coarse_cands()
    drs = np.unique(cands[:, 2])  # 16 rotations
    by_dr = {float(dr): np.where(cands[:, 2] == dr)[0] for dr in drs}

    # build device inputs: 16 theta-units per pair, 4 per core; cores 0-3
    # pair0, cores 4-7 pair1
    use_device = device_ok
    unit_map = {}  # (core, slot) -> (pair_idx, dr)
    in_maps = None
    if use_device:
        zero_nrq = np.zeros((KI, 2, NI, MMQ), np.float32)
        zero_st = np.zeros((KI, 2, G, RP, PF), np.float32)
        in_maps = []
        splat_fail = False
        nrq_cache = {}
        for core in range(N_CORES):
            pi = core // 4
            if pi >= len(pair_data):
                in_maps.append({"nrq": zero_nrq, "st": zero_st})
                continue
            pd = pair_data[pi]
            S_list = []
            for slot in range(U):
                th_idx = 4 * (core % 4) + slot
                dr = float(drs[th_idx])
                ix, iy = _theta_warp_fields(pd["base"], dr, ac)
                S = _build_splats(pd["ego_c"], ix, iy)
                if S is None:
                    splat_fail = True
                    break
                S_list.append(S)
                unit_map[(core, slot)] = (pi, dr)
            if splat_fail:
                break
            if pi not in nrq_cache:
                nrq_cache[pi] = _build_nrq(pd["nbr_c"])
            in_maps.append({"nrq": nrq_cache[pi], "st": _build_st(S_list)})
        if splat_fail:
            use_device = False

    if use_device:
        try:
            touts = _run_device(in_maps)
        except Exception:
            use_device = False

    for pi, pd in enumerate(pair_data):
        base = pd["base"]
        pair_device = use_device
        approx = None
        if pair_device:
            approx = np.empty(len(cands), np.float32)
            for core in range(4 * pi, 4 * pi + 4):
                for slot in range(U):
                    key = (core, slot)
                    if key not in unit_map:
                        continue
                    _, dr = unit_map[key]
                    sel = by_dr[dr]
                    a = _assemble_approx(touts[core][slot], base, cands[sel], ac)
                    if a is None:
                        pair_device = False
                        break
                    approx[sel] = a
                if not pair_device:
                    break
        if pair_device:
            cp = _finish_pair(pd["ego_c"], pd["nbrP"], base, cands, approx, ac)
            if np.all(cp == 0.0):
                new_aff = base
            else:
                fc = _fine_cands(cp)
                affs_f = _cand_affines(fc, base)
                sf = _exact_scores(pd["ego_c"], pd["nbrP"], affs_f, ac)
                bif = int(np.argmax(sf))
                new_aff = affs_f[bif] if sf[bif] > 1e-5 else base
        else:
            new_aff = _refine_pair_host_only(pd["ego_c"], pd["nbr_c"], base, ac)
        if pd["n"] < refined.shape[2] and pd["b"] < refined.shape[0]:
            refined[pd["b"], 0, pd["n"]] = new_aff

    return refined.astype(out_dtype)


# revision 11
# speedup vs baseline: 1.5871x; 1.2568x over previous
"""Trainium2 Bass kernel for nn_CartographerPoseCorrector (v2: fp8 DoubleRow).

Same algorithm as v1 (moment-correlation surfaces on TensorE + exact host
rescore of a margin set), restructured for speed:

- fp8e4m3 inputs + DoubleRow matmuls: contraction pairs the two halves of
  the 176-column canvas window, so one matmul contracts all columns.
- Lag windows trimmed to the candidate range actually reachable
  (48 x 48 lags, J-window 49).
- The (theta', m) axes are interleaved *inside* the canvas-row axis of the
  splat tensor, so each accumulation step's J-window is one contiguous
  392-element slice -> a legal 3-dim DoubleRow rhs AP.
- The nbr window tensor arrives from the host already repacked
  (no on-chip DVE repack).
- Dummy warm-up matmuls run during the input DMA to lift the PE HAM
  clock-gate before the real accumulation starts.

Geometry (device):
  W[ki, half, i, 2t+slot] = nbr[2i+slot, c + t - 68],  c = 21 + ki + 88*half
  X[ki, half, g, r', 4*tp+m] = S_{2g+tp}[m, 195 - r', c]
  psum[g][2t+slot, j*8 + tp*4 + m]
      += sum_{ki,half,i} W[ki,half,i,2t+slot] * X[ki,half,g,126-2i+j, ...]
  => T_m,theta[K=t-24, J=j+slot-25] after the host fold.
"""

import math
import sys

import numpy as np

H = W = 128
THRESH = 0.3
TRANS_RANGE = 20.0
ROT_RANGE = 15.0
COARSE_STEP = 2.0
FINE_STEP = 0.5

# Device-kernel geometry (must match the Bass program)
CANVAS = 224     # splat canvas extent (host-side bounds check)
OFF = 44         # image coord -> canvas coord offset
NL = 48          # lags per axis
LMIN = -24       # lag range [LMIN, LMIN + NL)
NJ = NL + 1      # J-window width per slot
MMQ = 2 * NL     # weight columns (= psum partitions)
KI = 88          # contraction partitions (x2 halves = 176 canvas cols)
CL0 = 21         # first canvas column on device
XOFF2 = 68       # nbr col = c + t - XOFF2
R0 = 126         # window offset: roff = R0 - 2*i
ROWBASE = 195    # S row = ROWBASE - r'
RP = 176         # r' extent
G = 2            # theta groups per core (2 thetas each)
PF = 8           # (theta', m) interleave factor inside r'
NI = H // 2      # accumulation steps
M = 4            # moments
U = 4            # thetas per core
N_CORES = 8
NFREE = NJ * PF  # 392: psum free size

DELTA_COARSE = 320.0   # exact-rescore safety margin
RESCORE_CAP = 2800     # hard cap on rescored coarse candidates per pair

_NC = None
LAST_MAPS = None  # debug/timing aid: last device input maps (fp8-converted)
LAST_TOUTS = None  # debug aid: last raw device outputs


# ----------------------------------------------------------------------------
# host math (mirrors reference numerics in fp32 where it matters)
# ----------------------------------------------------------------------------

def _grid_1d(align_corners):
    if align_corners:
        xs = np.linspace(-1.0, 1.0, W, dtype=np.float32)
        ys = np.linspace(-1.0, 1.0, H, dtype=np.float32)
    else:
        xs = ((2.0 * np.arange(W, dtype=np.float32) + 1.0) / W - 1.0)
        ys = ((2.0 * np.arange(H, dtype=np.float32) + 1.0) / H - 1.0)
    return xs, ys


def _coarse_cands():
    dxs = np.arange(-TRANS_RANGE, TRANS_RANGE + 1e-3, COARSE_STEP, dtype=np.float32)
    drs = np.arange(-ROT_RANGE, ROT_RANGE + 1e-3, COARSE_STEP, dtype=np.float32)
    gdx, gdy, gdr = np.meshgrid(dxs, dxs, drs, indexing="ij")
    return np.stack([gdx.ravel(), gdy.ravel(), gdr.ravel()], axis=1)


def _fine_cands(cp):
    off = np.arange(-COARSE_STEP, COARSE_STEP + 1e-3, FINE_STEP, dtype=np.float32)
    gdx, gdy, gdr = np.meshgrid(cp[0] + off, cp[1] + off, cp[2] + off, indexing="ij")
    return np.stack([gdx.ravel(), gdy.ravel(), gdr.ravel()], axis=1)


def _cand_affines(cands, base_2x3):
    dx, dy, dr = cands[:, 0], cands[:, 1], cands[:, 2]
    tx = (2.0 * dx / max(W - 1, 1)).astype(np.float32)
    ty = (2.0 * dy / max(H - 1, 1)).astype(np.float32)
    th = (dr * np.float32(math.pi / 180.0)).astype(np.float32)
    c, s = np.cos(th), np.sin(th)
    z, o = np.zeros_like(c), np.ones_like(c)
    delta = np.stack([c, -s, tx, s, c, ty, z, z, o], axis=-1).reshape(-1, 3, 3)
    base3 = np.concatenate([base_2x3, np.array([[0, 0, 1]], np.float32)], axis=0)
    return np.einsum("ij,njk->nik", base3.astype(np.float32), delta.astype(np.float32))[
        :, :2, :
    ].astype(np.float32)


def _pad_nbr(nbr_c, padb=8):
    out = np.zeros((H + 2 * padb, W + 2 * padb), np.float32)
    out[padb : padb + H, padb : padb + W] = nbr_c
    return out


def _exact_scores(ego_c, nbrP, affs, align_corners, padb=8, chunk=16):
    """Exact fp32 bilinear grid-sample scores for candidate affines [n,2,3]."""
    xs, ys = _grid_1d(align_corners)
    gx = np.broadcast_to(xs[None, :], (H, W)).ravel().astype(np.float32)
    gy = np.broadcast_to(ys[:, None], (H, W)).ravel().astype(np.float32)
    flat = nbrP.ravel()
    Wp = nbrP.shape[1]
    if align_corners:
        scx, ox = np.float32(0.5 * (W - 1)), np.float32(0.5 * (W - 1))
        scy, oy = np.float32(0.5 * (H - 1)), np.float32(0.5 * (H - 1))
    else:
        scx, ox = np.float32(0.5 * W), np.float32(0.5 * W - 0.5)
        scy, oy = np.float32(0.5 * H), np.float32(0.5 * H - 0.5)
    ego_f = ego_c.ravel().astype(np.float32)
    N = len(affs)
    out = np.empty(N, np.float32)
    for s0 in range(0, N, chunk):
        A = affs[s0 : s0 + chunk].astype(np.float32)
        n = len(A)
        ix = np.multiply.outer(A[:, 0, 0], gx)
        ix += np.multiply.outer(A[:, 0, 1], gy)
        ix += A[:, 0, 2, None]
        ix *= scx
        ix += ox
        iy = np.multiply.outer(A[:, 1, 0], gx)
        iy += np.multiply.outer(A[:, 1, 1], gy)
        iy += A[:, 1, 2, None]
        iy *= scy
        iy += oy
        x0 = np.floor(ix)
        y0 = np.floor(iy)
        wx = ix - x0
        wy = iy - y0
        xi = x0.astype(np.int32)
        xi += padb
        np.clip(xi, 0, Wp - 2, out=xi)
        yi = y0.astype(np.int32)
        yi += padb
        np.clip(yi, 0, Wp - 2, out=yi)
        base = yi
        base *= Wp
        base += xi
        b00 = flat[base]
        b01 = flat[base + 1]
        b10 = flat[base + Wp]
        b11 = flat[base + Wp + 1]
        top = (1.0 - wx) * b00
        top += wx * b01
        bot = (1.0 - wx) * b10
        bot += wx * b11
        val = (1.0 - wy) * top
        val += wy * bot
        out[s0 : s0 + n] = val @ ego_f
    return out


def _theta_warp_fields(base_2x3, dr, align_corners):
    """Pixel-coord sample positions of the theta-only warp (dx=dy=0)."""
    th = np.float32(dr) * np.float32(math.pi / 180.0)
    c, s = np.cos(th, dtype=np.float32), np.sin(th, dtype=np.float32)
    delta = np.array([[c, -s, 0], [s, c, 0], [0, 0, 1]], np.float32)
    base3 = np.concatenate([base_2x3, [[0, 0, 1]]], 0).astype(np.float32)
    aff = (base3 @ delta)[:2]
    xs, ys = _grid_1d(align_corners)
    gx = aff[0, 0] * xs[None, :] + aff[0, 1] * ys[:, None] + aff[0, 2]
    gy = aff[1, 0] * xs[None, :] + aff[1, 1] * ys[:, None] + aff[1, 2]
    if align_corners:
        ix = (gx + 1.0) * (0.5 * (W - 1))
        iy = (gy + 1.0) * (0.5 * (H - 1))
    else:
        ix = gx * (0.5 * W) + (0.5 * W - 0.5)
        iy = gy * (0.5 * H) + (0.5 * H - 0.5)
    return ix.astype(np.float32), iy.astype(np.float32)


def _trans_shifts(base_2x3, cands, align_corners):
    """Pixel-space shifts (ux, uy) each candidate translation adds."""
    B2 = base_2x3[:2, :2].astype(np.float32)
    tx = (2.0 * cands[:, 0] / (W - 1)).astype(np.float32)
    ty = (2.0 * cands[:, 1] / (H - 1)).astype(np.float32)
    if align_corners:
        sx, sy = 0.5 * (W - 1), 0.5 * (H - 1)
    else:
        sx, sy = 0.5 * W, 0.5 * H
    ux = (B2[0, 0] * tx + B2[0, 1] * ty) * np.float32(sx)
    uy = (B2[1, 0] * tx + B2[1, 1] * ty) * np.float32(sy)
    return ux, uy


def _build_splats(ego_c, ix, iy):
    """Moment splat canvases [4, CANVAS, CANVAS] f32, or None if out of range."""
    Xi = np.floor(ix)
    Yi = np.floor(iy)
    Xf = (ix - Xi).astype(np.float32)
    Yf = (iy - Yi).astype(np.float32)
    Xi = Xi.astype(np.int64)
    Yi = Yi.astype(np.int64)
    if (
        Xi.min() < -OFF
        or Xi.max() >= CANVAS - OFF
        or Yi.min() < -OFF
        or Yi.max() >= CANVAS - OFF
    ):
        return None
    S = np.zeros((M, CANVAS, CANVAS), np.float32)
    flatidx = ((Yi + OFF) * CANVAS + (Xi + OFF)).ravel()
    nbins = CANVAS * CANVAS
    for m, mu in enumerate((None, Xf, Yf, Xf * Yf)):
        wgt = ego_c if mu is None else mu * ego_c
        S[m] = (
            np.bincount(flatidx, weights=wgt.ravel().astype(np.float64), minlength=nbins)
            .reshape(CANVAS, CANVAS)
            .astype(np.float32)
        )
    return S


def _assemble_approx(T, base_2x3, cands, align_corners):
    """Approx scores for one theta's candidates from its surface T [NL, M, NL].

    Returns None if any candidate's lag falls outside the computed window
    (caller falls back to the exact host path)."""
    ux, uy = _trans_shifts(base_2x3, cands, align_corners)
    Ui = np.floor(ux).astype(np.int64)
    Ufx = (ux - Ui).astype(np.float32)
    Vi = np.floor(uy).astype(np.int64)
    Ufy = (uy - Vi).astype(np.float32)
    if (
        Ui.min() < LMIN
        or Ui.max() + 1 >= LMIN + NL
        or Vi.min() < LMIN
        or Vi.max() + 1 >= LMIN + NL
    ):
        return None
    out = np.zeros(len(cands), np.float32)
    for j in (0, 1):
        ay = np.where(j, Ufy, 1.0 - Ufy).astype(np.float32)
        by = 1.0 if j else -1.0
        Jp = Vi + j - LMIN
        for k in (0, 1):
            ax = np.where(k, Ufx, 1.0 - Ufx).astype(np.float32)
            bx = 1.0 if k else -1.0
            Kp = Ui + k - LMIN
            out += ax * ay * T[Kp, 0, Jp]
            out += bx * ay * T[Kp, 1, Jp]
            out += ax * by * T[Kp, 2, Jp]
            out += bx * by * T[Kp, 3, Jp]
    return out


def _build_nrq(nbr_c):
    """Host-side repacked window tensor [KI, NI, 2, MMQ] fp32.

    nrq[ki, i, half, 2t+slot] = nbr_c[2i+slot, (21 + ki + 88*half) + t - 68]
    with zero padding outside [0, W).  (i-major so the device can DMA it in
    independent i-chunks; half is the DoubleRow pair dim.)"""
    # padded transposed nbr: P[xp, y] = nbr[y, xp - 47], xp in [0, 224)
    P = np.zeros((KI + 88 + NL - 1 + 1, H), np.float32)  # 224 x 128
    P[47 : 47 + W, :] = nbr_c.T
    s0, s1 = P.strides
    A = np.lib.stride_tricks.as_strided(
        P, shape=(KI, 2, NL, H), strides=(s0, 88 * s0, s0, s1)
    )
    # [ki, half, t, y] -> [ki, i, half, 2t+slot]
    A5 = A.reshape(KI, 2, NL, NI, 2)           # [ki, half, t, i, slot]
    out = np.ascontiguousarray(A5.transpose(0, 3, 1, 2, 4)).reshape(KI, NI, 2, MMQ)
    return out


def _build_st(S_list):
    """Device splat tensor [KI, 2, G, RP, PF] fp32 from 4 splat canvases.

    st[ki, half, g, r', 4*tp + m] = S_{2g+tp}[m, ROWBASE - r', 21 + ki + 88*half]
    """
    st = np.zeros((KI, 2, G, RP, PF), np.float32)
    for th in range(U):
        g, tp = divmod(th, 2)
        S = S_list[th]
        for m in range(M):
            # rows ROWBASE..ROWBASE-RP+1, cols CL0..CL0+176
            Srev = S[m, ROWBASE : ROWBASE - RP : -1, CL0 : CL0 + 2 * KI]  # [RP, 176]
            cv = Srev.T.reshape(2, KI, RP)  # [half, ki, r']
            st[:, :, g, :, 4 * tp + m] = cv.transpose(1, 0, 2)
    return st


def _fold_tout(tout):
    """Device output [G*MMQ, NFREE] f32 -> per-theta surfaces [U, NL, M, NL]."""
    t5 = tout.reshape(G, MMQ, NJ, 2, M)  # [g, 2t+slot, j, tp, m]
    out = np.empty((U, NL, M, NL), np.float32)
    for th in range(U):
        g, tp = divmod(th, 2)
        a0 = t5[g, 0::2, :, tp, :]  # [t, j, m]
        a1 = t5[g, 1::2, :, tp, :]
        Tf = a0[:, 1 : 1 + NL, :] + a1[:, 0:NL, :]  # [t, J, m]
        out[th] = Tf.transpose(0, 2, 1)  # [K, m, J]
    return out


# ----------------------------------------------------------------------------
# device program
# ----------------------------------------------------------------------------

def _get_nc():
    global _NC
    if _NC is not None:
        return _NC
    sys.path.insert(0, "/opt/trn_rl_repo")
    from contextlib import ExitStack

    import concourse.bass as bass
    import concourse.mybir as mybir
    import concourse.tile as tile
    from concourse import bacc

    fp8 = mybir.dt.float8e4
    nc = bacc.Bacc("TRN2", target_bir_lowering=False, debug=False)
    nrq = nc.declare_dram_parameter("nrq", [KI, NI * 2 * MMQ], fp8, isOutput=False)
    st = nc.declare_dram_parameter("st", [KI, 2 * G * RP * PF], fp8, isOutput=False)
    tout = nc.declare_dram_parameter(
        "tout", [G * MMQ, NFREE], mybir.dt.float32, isOutput=True
    )
    nrq_h = nrq.tensor if isinstance(nrq, bass.AP) else nrq
    st_h = st.tensor if isinstance(st, bass.AP) else st
    tout_h = tout.tensor if isinstance(tout, bass.AP) else tout

    DR = mybir.MatmulPerfMode.DoubleRow
    NCHUNK = 4
    CI = NI // NCHUNK            # 16 i-steps per nrq chunk
    CB = CI * 2 * MMQ            # 3072 B per partition per chunk
    with ExitStack() as ctx:
        tc = ctx.enter_context(tile.TileContext(nc))
        pool = ctx.enter_context(tc.tile_pool(name="persist", bufs=1))
        psum_pool = ctx.enter_context(tc.tile_pool(name="psum", bufs=2, space="PSUM"))

        st_t = pool.tile([KI, 2 * G * RP * PF], fp8)
        nrq_c = [pool.tile([KI, CB], fp8, name=f"nrqc{k}", tag=f"nrqc{k}") for k in range(NCHUNK)]
        warm_sb = pool.tile([128, 512], fp8)

        # PE warm-up: dummy matmuls with no DMA dependency keep the PE busy
        # during the input DMA so the HAM clock-gate opens before the real
        # accumulation starts.
        warm_ps = psum_pool.tile([128, 512], mybir.dt.float32, name="warm", tag="warm")
        nc.vector.memset(warm_sb[:], 0)
        for _ in range(18):
            nc.tensor.matmul(
                warm_ps[:], warm_sb[:, 0:128], warm_sb[:], start=True, stop=True
            )

        # inputs spread over the three DMA paths (sync/scalar HWDGE queues,
        # gpsimd SWDGE) so the per-queue per-partition packet streams overlap
        nc.sync.dma_start(
            out=st_t[:],
            in_=bass.AP(tensor=st_h, offset=0,
                        ap=[[2 * G * RP * PF, KI], [1, 2 * G * RP * PF]]),
        )
        for k in range(NCHUNK):
            eng = nc.scalar if k < 2 else nc.gpsimd
            eng.dma_start(
                out=nrq_c[k][:],
                in_=bass.AP(tensor=nrq_h, offset=k * CB,
                            ap=[[NI * 2 * MMQ, KI], [1, CB]]),
            )

        psums = [
            psum_pool.tile([MMQ, NFREE], mybir.dt.float32, name=f"ps{g}", tag=f"ps{g}")
            for g in range(G)
        ]
        sb = st_t[:]
        HW_ST = G * RP * PF         # 2816: half stride in st tile
        for i in range(NI):
            wb = nrq_c[i // CI][:]
            Wap = bass.AP(
                tensor=wb.tensor,
                offset=wb.offset + (i % CI) * (2 * MMQ),
                ap=[list(wb.ap[0]), [MMQ, 2], [1, MMQ]],
            )
            if i % 4 == 3:
                # non-DR matmul: DoubleRow streams do not register as
                # PE-activity for the HAM clock gate; this keeps K=8/8.
                nc.tensor.matmul(
                    warm_ps[:, 0:128], warm_sb[:, 0:128], warm_sb[:, 0:128],
                    start=True, stop=True,
                )
            roff = (R0 - 2 * i) * PF
            for g in range(G):
                Xap = bass.AP(
                    tensor=sb.tensor,
                    offset=sb.offset + g * (RP * PF) + roff,
                    ap=[list(sb.ap[0]), [HW_ST, 2], [1, NFREE]],
                )
                nc.tensor.matmul(
                    psums[g][:],
                    Wap,
                    Xap,
                    start=(i == 0),
                    stop=(i == NI - 1),
                    perf_mode=DR,
                )

        stage_pool = ctx.enter_context(tc.tile_pool(name="stage", bufs=2))
        for g in range(G):
            stg = stage_pool.tile(
                [MMQ, NFREE], mybir.dt.float32, name=f"stg{g}", tag=f"stg{g}"
            )
            nc.vector.tensor_copy(stg[:], psums[g][:])
            dst = bass.AP(
                tensor=tout_h,
                offset=g * MMQ * NFREE,
                ap=[[NFREE, MMQ], [1, NFREE]],
            )
            nc.sync.dma_start(out=dst, in_=stg[:])
    nc.compile()
    _NC = nc
    return nc


def _run_device(in_maps):
    sys.path.insert(0, "/opt/trn_rl_repo")
    import ml_dtypes
    from concourse.bass_utils import run_bass_kernel_spmd

    maps = [
        {
            "nrq": np.ascontiguousarray(m["nrq"].reshape(KI, -1)).astype(
                ml_dtypes.float8_e4m3
            ),
            "st": np.ascontiguousarray(m["st"].reshape(KI, -1)).astype(
                ml_dtypes.float8_e4m3
            ),
        }
        for m in in_maps
    ]
    global LAST_MAPS, LAST_TOUTS
    LAST_MAPS = maps
    res = run_bass_kernel_spmd(_get_nc(), maps, core_ids=list(range(len(maps))))
    LAST_TOUTS = [r["tout"].astype(np.float32) for r in res.results]
    return [_fold_tout(t) for t in LAST_TOUTS]


# ----------------------------------------------------------------------------
# pipeline
# ----------------------------------------------------------------------------

def _refine_pair_host_only(ego_c, nbr_c, base, align_corners):
    """Pure-host exact fallback (pathological inputs only)."""
    nbrP = _pad_nbr(nbr_c)
    cands = _coarse_cands()
    sc = _exact_scores(ego_c, nbrP, _cand_affines(cands, base), align_corners)
    bi = int(np.argmax(sc))
    cp = cands[bi] if sc[bi] > 1e-5 else np.zeros(3, np.float32)
    if np.all(cp == 0.0):
        return base
    fc = _fine_cands(cp)
    affs_f = _cand_affines(fc, base)
    sf = _exact_scores(ego_c, nbrP, affs_f, align_corners)
    bif = int(np.argmax(sf))
    return affs_f[bif] if sf[bif] > 1e-5 else base


def _finish_pair(ego_c, nbrP, base, cands, approx, align_corners):
    """Adaptive exact rescore of the approx-selected coarse set -> cp."""
    thresh = approx.max() - DELTA_COARSE
    sel = np.where(approx >= thresh)[0]
    if len(sel) > RESCORE_CAP:
        sel = sel[np.argsort(approx[sel])[::-1][:RESCORE_CAP]]
    if len(sel) < 48:
        sel = np.argsort(approx)[::-1][:48]
    affs = _cand_affines(cands[sel], base)
    sc = _exact_scores(ego_c, nbrP, affs, align_corners)
    bi_local = int(np.argmax(sc))
    bi = int(sel[bi_local])
    ok = sc[bi_local] > 1e-5
    cp = cands[bi] if ok else np.zeros(3, np.float32)
    return cp


def kernel(occ_map, record_len, affine_matrix, align_corners):
    occ = np.asarray(occ_map, dtype=np.float32)
    rl = np.asarray(record_len).reshape(-1)
    aff_in = np.asarray(affine_matrix)
    out_dtype = aff_in.dtype
    refined = aff_in.astype(np.float32).copy()
    ac = bool(np.asarray(align_corners))

    # pair list exactly as the reference builds it
    pairs = []
    idx = 0
    for b in range(len(rl)):
        n_agents = int(rl[b])
        grp0 = idx
        idx += n_agents
        if n_agents <= 1:
            continue
        for n in range(1, n_agents):
            pairs.append((b, n, grp0, grp0 + n))
    if not pairs:
        return refined.astype(out_dtype)

    device_ok = (
        len(pairs) <= 2
        and all(
            b < refined.shape[0] and n < refined.shape[2] and nb < occ.shape[0]
            for (b, n, _, nb) in pairs
        )
    )

    pair_data = []
    for (b, n, ei, ni) in pairs:
        # mimic jax OOB semantics: clip gather indices, drop OOB scatters
        ei = min(ei, occ.shape[0] - 1)
        ni = min(ni, occ.shape[0] - 1)
        ego = occ[ei, 0]
        nbr = occ[ni, 0]
        ego_c = np.where(ego > THRESH, ego, 0.0).astype(np.float32)
        nbr_c = np.where(nbr > THRESH, nbr, 0.0).astype(np.float32)
        base = refined[b, 0, n].astype(np.float32)
        pair_data.append(
            {
                "b": min(b, refined.shape[0] - 1),
                "n": n,
                "ego_c": ego_c,
                "nbr_c": nbr_c,
                "nbrP": _pad_nbr(nbr_c),
                "base": base,
            }
        )

    cands = _coarse_cands()
    drs = np.unique(cands[:, 2])  # 16 rotations
    by_dr = {float(dr): np.where(cands[:, 2] == dr)[0] for dr in drs}

    # build device inputs: 16 theta-units per pair, 4 per core; cores 0-3
    # pair0, cores 4-7 pair1
    use_device = device_ok
    unit_map = {}  # (core, slot) -> (pair_idx, dr)
    in_maps = None
    if use_device:
        zero_nrq = np.zeros((KI, 2, NI, MMQ), np.float32)
        zero_st = np.zeros((KI, 2, G, RP, PF), np.float32)
        in_maps = []
        splat_fail = False
        nrq_cache = {}
        for core in range(N_CORES):
            pi = core // 4
            if pi >= len(pair_data):
                in_maps.append({"nrq": zero_nrq, "st": zero_st})
                continue
            pd = pair_data[pi]
            S_list = []
            for slot in range(U):
                th_idx = 4 * (core % 4) + slot
                dr = float(drs[th_idx])
                ix, iy = _theta_warp_fields(pd["base"], dr, ac)
                S = _build_splats(pd["ego_c"], ix, iy)
                if S is None:
                    splat_fail = True
                    break
                S_list.append(S)
                unit_map[(core, slot)] = (pi, dr)
            if splat_fail:
                break
            if pi not in nrq_cache:
                nrq_cache[pi] = _build_nrq(pd["nbr_c"])
            in_maps.append({"nrq": nrq_cache[pi], "st": _build_st(S_list)})
        if splat_fail:
            use_device = False

    if use_device:
        try:
            touts = _run_device(in_maps)
        except Exception:
            use_device = False

    for pi, pd in enumerate(pair_data):
        base = pd["base"]
        pair_device = use_device
        approx = None
        if pair_device:
            approx = np.empty(len(cands), np.float32)
            for core in range(4 * pi, 4 * pi + 4):
                for slot in range(U):
                    key = (core, slot)
                    if key not in unit_map:
                        continue
                    _, dr = unit_map[key]
                    sel = by_dr[dr]
                    a = _assemble_approx(touts[core][slot], base, cands[sel], ac)
                    if a is None:
                        pair_device = False
                        break
                    approx[sel] = a
                if not pair_device:
                    break
        if pair_device:
            cp = _finish_pair(pd["ego_c"], pd["nbrP"], base, cands, approx, ac)
            if np.all(cp == 0.0):
                new_aff = base
            else:
                fc = _fine_cands(cp)
                affs_f = _cand_affines(fc, base)
                sf = _exact_scores(pd["ego_c"], pd["nbrP"], affs_f, ac)
                bif = int(np.argmax(sf))
                new_aff = affs_f[bif] if sf[bif] > 1e-5 else base
        else:
            new_aff = _refine_pair_host_only(pd["ego_c"], pd["nbr_c"], base, ac)
        if pd["n"] < refined.shape[2] and pd["b"] < refined.shape[0]:
            refined[pd["b"], 0, pd["n"]] = new_aff

    return refined.astype(out_dtype)


# revision 12
# speedup vs baseline: 1.6208x; 1.0213x over previous
"""Trainium2 Bass kernel for nn_CartographerPoseCorrector (v2: fp8 DoubleRow).

Same algorithm as v1 (moment-correlation surfaces on TensorE + exact host
rescore of a margin set), restructured for speed:

- fp8e4m3 inputs + DoubleRow matmuls: contraction pairs the two halves of
  the 176-column canvas window, so one matmul contracts all columns.
- Lag windows trimmed to the candidate range actually reachable
  (48 x 48 lags, J-window 49).
- The (theta', m) axes are interleaved *inside* the canvas-row axis of the
  splat tensor, so each accumulation step's J-window is one contiguous
  392-element slice -> a legal 3-dim DoubleRow rhs AP.
- The nbr window tensor arrives from the host already repacked
  (no on-chip DVE repack).
- Dummy warm-up matmuls run during the input DMA to lift the PE HAM
  clock-gate before the real accumulation starts.

Geometry (device):
  W[ki, half, i, 2t+slot] = nbr[2i+slot, c + t - 68],  c = 21 + ki + 88*half
  X[ki, half, g, r', 4*tp+m] = S_{2g+tp}[m, 195 - r', c]
  psum[g][2t+slot, j*8 + tp*4 + m]
      += sum_{ki,half,i} W[ki,half,i,2t+slot] * X[ki,half,g,126-2i+j, ...]
  => T_m,theta[K=t-24, J=j+slot-25] after the host fold.
"""

import math
import sys

import numpy as np

H = W = 128
THRESH = 0.3
TRANS_RANGE = 20.0
ROT_RANGE = 15.0
COARSE_STEP = 2.0
FINE_STEP = 0.5

# Device-kernel geometry (must match the Bass program)
CANVAS = 224     # splat canvas extent (host-side bounds check)
OFF = 44         # image coord -> canvas coord offset
NL = 48          # lags per axis
LMIN = -24       # lag range [LMIN, LMIN + NL)
NJ = NL + 1      # J-window width per slot
MMQ = 2 * NL     # weight columns (= psum partitions)
KI = 88          # contraction partitions (x2 halves = 176 canvas cols)
CL0 = 21         # first canvas column on device
XOFF2 = 68       # nbr col = c + t - XOFF2
R0 = 126         # window offset: roff = R0 - 2*i
ROWBASE = 195    # S row = ROWBASE - r'
RP = 176         # r' extent
G = 2            # theta groups per core (2 thetas each)
PF = 8           # (theta', m) interleave factor inside r'
NI = H // 2      # accumulation steps
M = 4            # moments
U = 4            # thetas per core
N_CORES = 8
NFREE = NJ * PF  # 392: psum free size

DELTA_COARSE = 320.0   # exact-rescore safety margin
RESCORE_CAP = 2800     # hard cap on rescored coarse candidates per pair

_NC = None
LAST_MAPS = None  # debug/timing aid: last device input maps (fp8-converted)
LAST_TOUTS = None  # debug aid: last raw device outputs


# ----------------------------------------------------------------------------
# host math (mirrors reference numerics in fp32 where it matters)
# ----------------------------------------------------------------------------

def _grid_1d(align_corners):
    if align_corners:
        xs = np.linspace(-1.0, 1.0, W, dtype=np.float32)
        ys = np.linspace(-1.0, 1.0, H, dtype=np.float32)
    else:
        xs = ((2.0 * np.arange(W, dtype=np.float32) + 1.0) / W - 1.0)
        ys = ((2.0 * np.arange(H, dtype=np.float32) + 1.0) / H - 1.0)
    return xs, ys


def _coarse_cands():
    dxs = np.arange(-TRANS_RANGE, TRANS_RANGE + 1e-3, COARSE_STEP, dtype=np.float32)
    drs = np.arange(-ROT_RANGE, ROT_RANGE + 1e-3, COARSE_STEP, dtype=np.float32)
    gdx, gdy, gdr = np.meshgrid(dxs, dxs, drs, indexing="ij")
    return np.stack([gdx.ravel(), gdy.ravel(), gdr.ravel()], axis=1)


def _fine_cands(cp):
    off = np.arange(-COARSE_STEP, COARSE_STEP + 1e-3, FINE_STEP, dtype=np.float32)
    gdx, gdy, gdr = np.meshgrid(cp[0] + off, cp[1] + off, cp[2] + off, indexing="ij")
    return np.stack([gdx.ravel(), gdy.ravel(), gdr.ravel()], axis=1)


def _cand_affines(cands, base_2x3):
    dx, dy, dr = cands[:, 0], cands[:, 1], cands[:, 2]
    tx = (2.0 * dx / max(W - 1, 1)).astype(np.float32)
    ty = (2.0 * dy / max(H - 1, 1)).astype(np.float32)
    th = (dr * np.float32(math.pi / 180.0)).astype(np.float32)
    c, s = np.cos(th), np.sin(th)
    z, o = np.zeros_like(c), np.ones_like(c)
    delta = np.stack([c, -s, tx, s, c, ty, z, z, o], axis=-1).reshape(-1, 3, 3)
    base3 = np.concatenate([base_2x3, np.array([[0, 0, 1]], np.float32)], axis=0)
    return np.einsum("ij,njk->nik", base3.astype(np.float32), delta.astype(np.float32))[
        :, :2, :
    ].astype(np.float32)


def _pad_nbr(nbr_c, padb=8):
    out = np.zeros((H + 2 * padb, W + 2 * padb), np.float32)
    out[padb : padb + H, padb : padb + W] = nbr_c
    return out


def _exact_scores(ego_c, nbrP, affs, align_corners, padb=8, chunk=16):
    """Exact fp32 bilinear grid-sample scores for candidate affines [n,2,3]."""
    xs, ys = _grid_1d(align_corners)
    gx = np.broadcast_to(xs[None, :], (H, W)).ravel().astype(np.float32)
    gy = np.broadcast_to(ys[:, None], (H, W)).ravel().astype(np.float32)
    flat = nbrP.ravel()
    Wp = nbrP.shape[1]
    if align_corners:
        scx, ox = np.float32(0.5 * (W - 1)), np.float32(0.5 * (W - 1))
        scy, oy = np.float32(0.5 * (H - 1)), np.float32(0.5 * (H - 1))
    else:
        scx, ox = np.float32(0.5 * W), np.float32(0.5 * W - 0.5)
        scy, oy = np.float32(0.5 * H), np.float32(0.5 * H - 0.5)
    ego_f = ego_c.ravel().astype(np.float32)
    N = len(affs)
    out = np.empty(N, np.float32)
    for s0 in range(0, N, chunk):
        A = affs[s0 : s0 + chunk].astype(np.float32)
        n = len(A)
        ix = np.multiply.outer(A[:, 0, 0], gx)
        ix += np.multiply.outer(A[:, 0, 1], gy)
        ix += A[:, 0, 2, None]
        ix *= scx
        ix += ox
        iy = np.multiply.outer(A[:, 1, 0], gx)
        iy += np.multiply.outer(A[:, 1, 1], gy)
        iy += A[:, 1, 2, None]
        iy *= scy
        iy += oy
        x0 = np.floor(ix)
        y0 = np.floor(iy)
        wx = ix - x0
        wy = iy - y0
        xi = x0.astype(np.int32)
        xi += padb
        np.clip(xi, 0, Wp - 2, out=xi)
        yi = y0.astype(np.int32)
        yi += padb
        np.clip(yi, 0, Wp - 2, out=yi)
        base = yi
        base *= Wp
        base += xi
        b00 = flat[base]
        b01 = flat[base + 1]
        b10 = flat[base + Wp]
        b11 = flat[base + Wp + 1]
        top = (1.0 - wx) * b00
        top += wx * b01
        bot = (1.0 - wx) * b10
        bot += wx * b11
        val = (1.0 - wy) * top
        val += wy * bot
        out[s0 : s0 + n] = val @ ego_f
    return out


def _theta_warp_fields(base_2x3, dr, align_corners):
    """Pixel-coord sample positions of the theta-only warp (dx=dy=0)."""
    th = np.float32(dr) * np.float32(math.pi / 180.0)
    c, s = np.cos(th, dtype=np.float32), np.sin(th, dtype=np.float32)
    delta = np.array([[c, -s, 0], [s, c, 0], [0, 0, 1]], np.float32)
    base3 = np.concatenate([base_2x3, [[0, 0, 1]]], 0).astype(np.float32)
    aff = (base3 @ delta)[:2]
    xs, ys = _grid_1d(align_corners)
    gx = aff[0, 0] * xs[None, :] + aff[0, 1] * ys[:, None] + aff[0, 2]
    gy = aff[1, 0] * xs[None, :] + aff[1, 1] * ys[:, None] + aff[1, 2]
    if align_corners:
        ix = (gx + 1.0) * (0.5 * (W - 1))
        iy = (gy + 1.0) * (0.5 * (H - 1))
    else:
        ix = gx * (0.5 * W) + (0.5 * W - 0.5)
        iy = gy * (0.5 * H) + (0.5 * H - 0.5)
    return ix.astype(np.float32), iy.astype(np.float32)


def _trans_shifts(base_2x3, cands, align_corners):
    """Pixel-space shifts (ux, uy) each candidate translation adds."""
    B2 = base_2x3[:2, :2].astype(np.float32)
    tx = (2.0 * cands[:, 0] / (W - 1)).astype(np.float32)
    ty = (2.0 * cands[:, 1] / (H - 1)).astype(np.float32)
    if align_corners:
        sx, sy = 0.5 * (W - 1), 0.5 * (H - 1)
    else:
        sx, sy = 0.5 * W, 0.5 * H
    ux = (B2[0, 0] * tx + B2[0, 1] * ty) * np.float32(sx)
    uy = (B2[1, 0] * tx + B2[1, 1] * ty) * np.float32(sy)
    return ux, uy


def _build_splats(ego_c, ix, iy):
    """Moment splat canvases [4, CANVAS, CANVAS] f32, or None if out of range."""
    Xi = np.floor(ix)
    Yi = np.floor(iy)
    Xf = (ix - Xi).astype(np.float32)
    Yf = (iy - Yi).astype(np.float32)
    Xi = Xi.astype(np.int64)
    Yi = Yi.astype(np.int64)
    if (
        Xi.min() < -OFF
        or Xi.max() >= CANVAS - OFF
        or Yi.min() < -OFF
        or Yi.max() >= CANVAS - OFF
    ):
        return None
    S = np.zeros((M, CANVAS, CANVAS), np.float32)
    flatidx = ((Yi + OFF) * CANVAS + (Xi + OFF)).ravel()
    nbins = CANVAS * CANVAS
    for m, mu in enumerate((None, Xf, Yf, Xf * Yf)):
        wgt = ego_c if mu is None else mu * ego_c
        S[m] = (
            np.bincount(flatidx, weights=wgt.ravel().astype(np.float64), minlength=nbins)
            .reshape(CANVAS, CANVAS)
            .astype(np.float32)
        )
    return S


def _assemble_approx(T, base_2x3, cands, align_corners):
    """Approx scores for one theta's candidates from its surface T [NL, M, NL].

    Returns None if any candidate's lag falls outside the computed window
    (caller falls back to the exact host path)."""
    ux, uy = _trans_shifts(base_2x3, cands, align_corners)
    Ui = np.floor(ux).astype(np.int64)
    Ufx = (ux - Ui).astype(np.float32)
    Vi = np.floor(uy).astype(np.int64)
    Ufy = (uy - Vi).astype(np.float32)
    if (
        Ui.min() < LMIN
        or Ui.max() + 1 >= LMIN + NL
        or Vi.min() < LMIN
        or Vi.max() + 1 >= LMIN + NL
    ):
        return None
    out = np.zeros(len(cands), np.float32)
    for j in (0, 1):
        ay = np.where(j, Ufy, 1.0 - Ufy).astype(np.float32)
        by = 1.0 if j else -1.0
        Jp = Vi + j - LMIN
        for k in (0, 1):
            ax = np.where(k, Ufx, 1.0 - Ufx).astype(np.float32)
            bx = 1.0 if k else -1.0
            Kp = Ui + k - LMIN
            out += ax * ay * T[Kp, 0, Jp]
            out += bx * ay * T[Kp, 1, Jp]
            out += ax * by * T[Kp, 2, Jp]
            out += bx * by * T[Kp, 3, Jp]
    return out


def _build_nrq(nbr_c):
    """Host-side repacked window tensor [KI, NI, 2, MMQ] fp32.

    nrq[ki, i, half, 2t+slot] = nbr_c[2i+slot, (21 + ki + 88*half) + t - 68]
    with zero padding outside [0, W).  (i-major so the device can DMA it in
    independent i-chunks; half is the DoubleRow pair dim.)"""
    # padded transposed nbr: P[xp, y] = nbr[y, xp - 47], xp in [0, 224)
    P = np.zeros((KI + 88 + NL - 1 + 1, H), np.float32)  # 224 x 128
    P[47 : 47 + W, :] = nbr_c.T
    s0, s1 = P.strides
    A = np.lib.stride_tricks.as_strided(
        P, shape=(KI, 2, NL, H), strides=(s0, 88 * s0, s0, s1)
    )
    # [ki, half, t, y] -> [ki, i, half, 2t+slot]
    A5 = A.reshape(KI, 2, NL, NI, 2)           # [ki, half, t, i, slot]
    out = np.ascontiguousarray(A5.transpose(0, 3, 1, 2, 4)).reshape(KI, NI, 2, MMQ)
    return out


def _build_st(S_list):
    """Device splat tensor [KI, 2, G, RP, PF] fp32 from 4 splat canvases.

    st[ki, half, g, r', 4*tp + m] = S_{2g+tp}[m, ROWBASE - r', 21 + ki + 88*half]
    """
    st = np.zeros((KI, 2, G, RP, PF), np.float32)
    for th in range(U):
        g, tp = divmod(th, 2)
        S = S_list[th]
        for m in range(M):
            # rows ROWBASE..ROWBASE-RP+1, cols CL0..CL0+176
            Srev = S[m, ROWBASE : ROWBASE - RP : -1, CL0 : CL0 + 2 * KI]  # [RP, 176]
            cv = Srev.T.reshape(2, KI, RP)  # [half, ki, r']
            st[:, :, g, :, 4 * tp + m] = cv.transpose(1, 0, 2)
    return st


def _fold_tout(tout):
    """Device output [G*MMQ, NFREE] f32 -> per-theta surfaces [U, NL, M, NL]."""
    t5 = tout.reshape(G, MMQ, NJ, 2, M)  # [g, 2t+slot, j, tp, m]
    out = np.empty((U, NL, M, NL), np.float32)
    for th in range(U):
        g, tp = divmod(th, 2)
        a0 = t5[g, 0::2, :, tp, :]  # [t, j, m]
        a1 = t5[g, 1::2, :, tp, :]
        Tf = a0[:, 1 : 1 + NL, :] + a1[:, 0:NL, :]  # [t, J, m]
        out[th] = Tf.transpose(0, 2, 1)  # [K, m, J]
    return out


# ----------------------------------------------------------------------------
# device program
# ----------------------------------------------------------------------------

def _get_nc():
    global _NC
    if _NC is not None:
        return _NC
    sys.path.insert(0, "/opt/trn_rl_repo")
    from contextlib import ExitStack

    import concourse.bass as bass
    import concourse.mybir as mybir
    import concourse.tile as tile
    from concourse import bacc

    fp8 = mybir.dt.float8e4
    nc = bacc.Bacc("TRN2", target_bir_lowering=False, debug=False)
    nrq = nc.declare_dram_parameter("nrq", [KI, NI * 2 * MMQ], fp8, isOutput=False)
    st = nc.declare_dram_parameter("st", [KI, 2 * G * RP * PF], fp8, isOutput=False)
    tout = nc.declare_dram_parameter(
        "tout", [G * MMQ, NFREE], mybir.dt.float32, isOutput=True
    )
    nrq_h = nrq.tensor if isinstance(nrq, bass.AP) else nrq
    st_h = st.tensor if isinstance(st, bass.AP) else st
    tout_h = tout.tensor if isinstance(tout, bass.AP) else tout

    DR = mybir.MatmulPerfMode.DoubleRow
    NCHUNK = 4
    CI = NI // NCHUNK            # 16 i-steps per nrq chunk
    CB = CI * 2 * MMQ            # 3072 B per partition per chunk
    with ExitStack() as ctx:
        tc = ctx.enter_context(tile.TileContext(nc))
        pool = ctx.enter_context(tc.tile_pool(name="persist", bufs=1))
        psum_pool = ctx.enter_context(tc.tile_pool(name="psum", bufs=2, space="PSUM"))

        st_t = pool.tile([KI, 2 * G * RP * PF], fp8)
        nrq_c = [pool.tile([KI, CB], fp8, name=f"nrqc{k}", tag=f"nrqc{k}") for k in range(NCHUNK)]
        warm_sb = pool.tile([128, 512], fp8)

        # PE warm-up: dummy matmuls with no DMA dependency keep the PE busy
        # during the input DMA so the HAM clock-gate opens before the real
        # accumulation starts.
        warm_ps = psum_pool.tile([128, 512], mybir.dt.float32, name="warm", tag="warm")
        nc.vector.memset(warm_sb[:], 0)
        for _ in range(12):
            nc.tensor.matmul(
                warm_ps[:], warm_sb[:, 0:128], warm_sb[:], start=True, stop=True
            )

        # inputs spread over the three DMA paths (sync/scalar HWDGE queues,
        # gpsimd SWDGE), ordered so the g=0 accumulation phase's deps
        # (st half 0, nrq chunk 0) land first.
        STH = RP * PF  # 1408: one g-half of st, per partition
        stv = st_t[:]

        def st_slice_dma(eng, g):
            dst = bass.AP(tensor=stv.tensor, offset=stv.offset + g * STH,
                          ap=[list(stv.ap[0]), [G * STH, 2], [1, STH]])
            src = bass.AP(tensor=st_h, offset=g * STH,
                          ap=[[2 * G * STH, KI], [G * STH, 2], [1, STH]])
            eng.dma_start(out=dst, in_=src)

        def nrq_dma(eng, k):
            eng.dma_start(
                out=nrq_c[k][:],
                in_=bass.AP(tensor=nrq_h, offset=k * CB,
                            ap=[[NI * 2 * MMQ, KI], [1, CB]]),
            )

        nrq_dma(nc.scalar, 0)
        st_slice_dma(nc.sync, 0)
        st_slice_dma(nc.scalar, 1)
        nrq_dma(nc.sync, 1)
        nrq_dma(nc.gpsimd, 2)
        nrq_dma(nc.gpsimd, 3)

        psums = [
            psum_pool.tile([MMQ, NFREE], mybir.dt.float32, name=f"ps{g}", tag=f"ps{g}")
            for g in range(G)
        ]
        stage_pool = ctx.enter_context(tc.tile_pool(name="stage", bufs=2))
        HW_ST = G * RP * PF         # 2816: half stride in st tile
        nmm = 0
        for g in range(G):
            for i in range(NI):
                wb = nrq_c[i // CI][:]
                Wap = bass.AP(
                    tensor=wb.tensor,
                    offset=wb.offset + (i % CI) * (2 * MMQ),
                    ap=[list(wb.ap[0]), [MMQ, 2], [1, MMQ]],
                )
                if nmm % 8 == 3:
                    # non-DR matmul: DoubleRow streams do not register as
                    # PE-activity for the HAM clock gate; this keeps K=8/8.
                    nc.tensor.matmul(
                        warm_ps[:, 0:128], warm_sb[:, 0:128], warm_sb[:, 0:128],
                        start=True, stop=True,
                    )
                nmm += 1
                Xap = bass.AP(
                    tensor=stv.tensor,
                    offset=stv.offset + g * (RP * PF) + (R0 - 2 * i) * PF,
                    ap=[list(stv.ap[0]), [HW_ST, 2], [1, NFREE]],
                )
                nc.tensor.matmul(
                    psums[g][:],
                    Wap,
                    Xap,
                    start=(i == 0),
                    stop=(i == NI - 1),
                    perf_mode=DR,
                )
            # g's result drains while the next g-phase computes
            stg = stage_pool.tile(
                [MMQ, NFREE], mybir.dt.float32, name=f"stg{g}", tag=f"stg{g}"
            )
            nc.vector.tensor_copy(stg[:], psums[g][:])
            dst = bass.AP(
                tensor=tout_h,
                offset=g * MMQ * NFREE,
                ap=[[NFREE, MMQ], [1, NFREE]],
            )
            nc.sync.dma_start(out=dst, in_=stg[:])
    nc.compile()
    _NC = nc
    return nc


def _run_device(in_maps):
    sys.path.insert(0, "/opt/trn_rl_repo")
    import ml_dtypes
    from concourse.bass_utils import run_bass_kernel_spmd

    maps = [
        {
            "nrq": np.ascontiguousarray(m["nrq"].reshape(KI, -1)).astype(
                ml_dtypes.float8_e4m3
            ),
            "st": np.ascontiguousarray(m["st"].reshape(KI, -1)).astype(
                ml_dtypes.float8_e4m3
            ),
        }
        for m in in_maps
    ]
    global LAST_MAPS, LAST_TOUTS
    LAST_MAPS = maps
    res = run_bass_kernel_spmd(_get_nc(), maps, core_ids=list(range(len(maps))))
    LAST_TOUTS = [r["tout"].astype(np.float32) for r in res.results]
    return [_fold_tout(t) for t in LAST_TOUTS]


# ----------------------------------------------------------------------------
# pipeline
# ----------------------------------------------------------------------------

def _refine_pair_host_only(ego_c, nbr_c, base, align_corners):
    """Pure-host exact fallback (pathological inputs only)."""
    nbrP = _pad_nbr(nbr_c)
    cands = _coarse_cands()
    sc = _exact_scores(ego_c, nbrP, _cand_affines(cands, base), align_corners)
    bi = int(np.argmax(sc))
    cp = cands[bi] if sc[bi] > 1e-5 else np.zeros(3, np.float32)
    if np.all(cp == 0.0):
        return base
    fc = _fine_cands(cp)
    affs_f = _cand_affines(fc, base)
    sf = _exact_scores(ego_c, nbrP, affs_f, align_corners)
    bif = int(np.argmax(sf))
    return affs_f[bif] if sf[bif] > 1e-5 else base


def _finish_pair(ego_c, nbrP, base, cands, approx, align_corners):
    """Adaptive exact rescore of the approx-selected coarse set -> cp."""
    thresh = approx.max() - DELTA_COARSE
    sel = np.where(approx >= thresh)[0]
    if len(sel) > RESCORE_CAP:
        sel = sel[np.argsort(approx[sel])[::-1][:RESCORE_CAP]]
    if len(sel) < 48:
        sel = np.argsort(approx)[::-1][:48]
    affs = _cand_affines(cands[sel], base)
    sc = _exact_scores(ego_c, nbrP, affs, align_corners)
    bi_local = int(np.argmax(sc))
    bi = int(sel[bi_local])
    ok = sc[bi_local] > 1e-5
    cp = cands[bi] if ok else np.zeros(3, np.float32)
    return cp


def kernel(occ_map, record_len, affine_matrix, align_corners):
    occ = np.asarray(occ_map, dtype=np.float32)
    rl = np.asarray(record_len).reshape(-1)
    aff_in = np.asarray(affine_matrix)
    out_dtype = aff_in.dtype
    refined = aff_in.astype(np.float32).copy()
    ac = bool(np.asarray(align_corners))

    # pair list exactly as the reference builds it
    pairs = []
    idx = 0
    for b in range(len(rl)):
        n_agents = int(rl[b])
        grp0 = idx
        idx += n_agents
        if n_agents <= 1:
            continue
        for n in range(1, n_agents):
            pairs.append((b, n, grp0, grp0 + n))
    if not pairs:
        return refined.astype(out_dtype)

    device_ok = (
        len(pairs) <= 2
        and all(
            b < refined.shape[0] and n < refined.shape[2] and nb < occ.shape[0]
            for (b, n, _, nb) in pairs
        )
    )

    pair_data = []
    for (b, n, ei, ni) in pairs:
        # mimic jax OOB semantics: clip gather indices, drop OOB scatters
        ei = min(ei, occ.shape[0] - 1)
        ni = min(ni, occ.shape[0] - 1)
        ego = occ[ei, 0]
        nbr = occ[ni, 0]
        ego_c = np.where(ego > THRESH, ego, 0.0).astype(np.float32)
        nbr_c = np.where(nbr > THRESH, nbr, 0.0).astype(np.float32)
        base = refined[b, 0, n].astype(np.float32)
        pair_data.append(
            {
                "b": min(b, refined.shape[0] - 1),
                "n": n,
                "ego_c": ego_c,
                "nbr_c": nbr_c,
                "nbrP": _pad_nbr(nbr_c),
                "base": base,
            }
        )

    cands = _coarse_cands()
    drs = np.unique(cands[:, 2])  # 16 rotations
    by_dr = {float(dr): np.where(cands[:, 2] == dr)[0] for dr in drs}

    # build device inputs: 16 theta-units per pair, 4 per core; cores 0-3
    # pair0, cores 4-7 pair1
    use_device = device_ok
    unit_map = {}  # (core, slot) -> (pair_idx, dr)
    in_maps = None
    if use_device:
        zero_nrq = np.zeros((KI, 2, NI, MMQ), np.float32)
        zero_st = np.zeros((KI, 2, G, RP, PF), np.float32)
        in_maps = []
        splat_fail = False
        nrq_cache = {}
        for core in range(N_CORES):
            pi = core // 4
            if pi >= len(pair_data):
                in_maps.append({"nrq": zero_nrq, "st": zero_st})
                continue
            pd = pair_data[pi]
            S_list = []
            for slot in range(U):
                th_idx = 4 * (core % 4) + slot
                dr = float(drs[th_idx])
                ix, iy = _theta_warp_fields(pd["base"], dr, ac)
                S = _build_splats(pd["ego_c"], ix, iy)
                if S is None:
                    splat_fail = True
                    break
                S_list.append(S)
                unit_map[(core, slot)] = (pi, dr)
            if splat_fail:
                break
            if pi not in nrq_cache:
                nrq_cache[pi] = _build_nrq(pd["nbr_c"])
            in_maps.append({"nrq": nrq_cache[pi], "st": _build_st(S_list)})
        if splat_fail:
            use_device = False

    if use_device:
        try:
            touts = _run_device(in_maps)
        except Exception:
            use_device = False

    for pi, pd in enumerate(pair_data):
        base = pd["base"]
        pair_device = use_device
        approx = None
        if pair_device:
            approx = np.empty(len(cands), np.float32)
            for core in range(4 * pi, 4 * pi + 4):
                for slot in range(U):
                    key = (core, slot)
                    if key not in unit_map:
                        continue
                    _, dr = unit_map[key]
                    sel = by_dr[dr]
                    a = _assemble_approx(touts[core][slot], base, cands[sel], ac)
                    if a is None:
                        pair_device = False
                        break
                    approx[sel] = a
                if not pair_device:
                    break
        if pair_device:
            cp = _finish_pair(pd["ego_c"], pd["nbrP"], base, cands, approx, ac)
            if np.all(cp == 0.0):
                new_aff = base
            else:
                fc = _fine_cands(cp)
                affs_f = _cand_affines(fc, base)
                sf = _exact_scores(pd["ego_c"], pd["nbrP"], affs_f, ac)
                bif = int(np.argmax(sf))
                new_aff = affs_f[bif] if sf[bif] > 1e-5 else base
        else:
            new_aff = _refine_pair_host_only(pd["ego_c"], pd["nbr_c"], base, ac)
        if pd["n"] < refined.shape[2] and pd["b"] < refined.shape[0]:
            refined[pd["b"], 0, pd["n"]] = new_aff

    return refined.astype(out_dtype)
